# revision 1
# baseline (speedup 1.0000x reference)
"""Trainium2 Bass kernel for nn_Decoder_MLP: Linear->BN->LIF -> Linear->BN->LIF.

Sharding: data-parallel over batch B (TB=T*B=128 rows -> 4 batch items/core,
all T=4 timesteps local). BN batch stats are all-reduced across the 8 cores.

Reference semantics replicated exactly, including the "scrambled" reshapes
(T,B,N,H)->(TB,H,N) which reinterpret (N,H) blocks as (H,N) row-major. That
scramble is handled by writing LIF-1 spikes to DRAM in (m, h) row-major order
(m = r*196+n) and reading them back with a strided access pattern as
(i, m') tiles, where i = the scrambled contraction index.

Layer-1 matmul runs in native fp32 on the PE (4 cyc/row) for precision near
the LIF threshold; layer-2 uses exact-bf16 spikes with w2 split into
bf16 hi+lo parts (two matmuls, ~fp32-quality product precision).
"""

import numpy as np
import ml_dtypes

import concourse.bass as bass
import concourse.mybir as mybir
import concourse.tile as tile
from concourse import bacc
from concourse.bass_utils import run_bass_kernel_spmd
from concourse.masks import make_identity

F32 = mybir.dt.float32
BF16 = mybir.dt.bfloat16
ALU = mybir.AluOpType
ACTF = mybir.ActivationFunctionType

N_CORES = 8
T = 4
B_GLOB = 32
B_LOC = B_GLOB // N_CORES          # 4 batch items per core
R = T * B_LOC                      # 16 local (t, b) rows
NN = 196                           # sequence/pixels dim N
C = 512
H = 2048
M = R * NN                         # 3136 local rows of the flattened GEMM
M_T = 392                          # = 2*NN, keeps m-tiles r-aligned
N_MT = M // M_T                    # 8
MB = 112                           # transpose block (3136 = 28*112)
N_MB = M // MB                     # 28
C_CHUNKS = C // 128                # 4
H_TILES = H // 128                 # 16
CT_TILES = C // 128                # 4
EPS = 1e-5
NTOT = float(B_GLOB * T // T * NN * T * B_GLOB // B_GLOB)  # placeholder, unused


def rne_keep(x, bits):
    """Round-to-nearest-even keeping `bits` explicit fp32 mantissa bits —
    bit-exact emulation of the PE's fp32r input rounding (measured RNE-11)."""
    u = np.ascontiguousarray(x, dtype=np.float32).view(np.uint32)
    shift = 23 - bits
    half = np.uint32(1 << (shift - 1))
    lsb = np.uint32(1 << shift)
    mask = np.uint32(~(lsb - np.uint32(1)))
    out = (u + half - np.uint32(1) + ((u >> np.uint32(shift)) & np.uint32(1))) & mask
    return out.view(np.float32)


def _emit_lif(nc, sb_h, sp_out, vpool, s_vecs, c_vecs, n_free, n_t=T):
    """BN-apply + LIF on a (channel 128, m) tile; m = (t, b, n) t-major.
    V_t = V'_{t-1} + 2^{t-1}*(scale*h_t + shift); spike iff V_t >= 2^t;
    V'_t = V_t * (V_t < 2^t)."""
    vprev = None
    for t in range(n_t):
        thr = float(2.0 ** (t + 1))
        hsl = sb_h[t] if isinstance(sb_h, list) else \
            sb_h[:, t * n_free:(t + 1) * n_free]
        if t == 0:
            v = vpool.tile([128, n_free], F32, tag="v")
            nc.scalar.activation(out=v, in_=hsl, func=ACTF.Identity,
                                 bias=c_vecs[t], scale=s_vecs[t])
        else:
            y = vpool.tile([128, n_free], F32, tag="y")
            nc.scalar.activation(out=y, in_=hsl, func=ACTF.Identity,
                                 bias=c_vecs[t], scale=s_vecs[t])
            v = vpool.tile([128, n_free], F32, tag="v")
            nc.vector.tensor_tensor(out=v, in0=vprev, in1=y, op=ALU.add)
        nc.gpsimd.tensor_scalar(out=sp_out[:, t * n_free:(t + 1) * n_free],
                                in0=v, scalar1=thr, scalar2=None, op0=ALU.is_ge)
        if t < n_t - 1:
            vp = vpool.tile([128, n_free], F32, tag="vp")
            nc.vector.scalar_tensor_tensor(out=vp, in0=v, scalar=thr, in1=v,
                                           op0=ALU.is_lt, op1=ALU.mult)
            vprev = vp


def _emit_stats_to_scales(nc, pool, ar_sb, gamma_sb, beta_sb, w, sfx):
    """ar_sb: (128, 2w) all-reduced [sum-of-means | sum-of-Ex2]. Returns per-t
    (s_vecs, c_vecs) lists of (128, w) tiles: 2^t*scale, 2^t*shift."""
    mean = pool.tile([128, w], F32, tag=f"bnmean{sfx}", name=f"bnmean{sfx}")
    ex2 = pool.tile([128, w], F32, tag=f"bnex2{sfx}", name=f"bnex2{sfx}")
    nc.vector.tensor_scalar(out=mean, in0=ar_sb[:, 0:w], scalar1=1.0 / N_CORES,
                            scalar2=None, op0=ALU.mult)
    nc.vector.tensor_scalar(out=ex2, in0=ar_sb[:, w:2 * w],
                            scalar1=1.0 / N_CORES, scalar2=None, op0=ALU.mult)
    var = pool.tile([128, w], F32, tag=f"bnvar{sfx}", name=f"bnvar{sfx}")
    msq = pool.tile([128, w], F32, tag=f"bnmsq{sfx}", name=f"bnmsq{sfx}")
    nc.vector.tensor_tensor(out=msq, in0=mean, in1=mean, op=ALU.mult)
    nc.vector.tensor_tensor(out=var, in0=ex2, in1=msq, op=ALU.subtract)
    epsb = pool.tile([128, 1], F32, tag=f"bneps{sfx}", name=f"bneps{sfx}")
    nc.vector.memset(epsb, EPS)
    std = pool.tile([128, w], F32, tag=f"bnstd{sfx}", name=f"bnstd{sfx}")
    nc.scalar.activation(out=std, in_=var, func=ACTF.Sqrt, bias=epsb, scale=1.0)
    rstd = pool.tile([128, w], F32, tag=f"bnrstd{sfx}", name=f"bnrstd{sfx}")
    nc.vector.reciprocal(out=rstd, in_=std)
    scale = pool.tile([128, w], F32, tag=f"bnscale{sfx}", name=f"bnscale{sfx}")
    nc.vector.tensor_tensor(out=scale, in0=gamma_sb, in1=rstd, op=ALU.mult)
    mscl = pool.tile([128, w], F32, tag=f"bnmscl{sfx}", name=f"bnmscl{sfx}")
    nc.vector.tensor_tensor(out=mscl, in0=mean, in1=scale, op=ALU.mult)
    shift = pool.tile([128, w], F32, tag=f"bnshift{sfx}", name=f"bnshift{sfx}")
    nc.vector.tensor_tensor(out=shift, in0=beta_sb, in1=mscl, op=ALU.subtract)
    s_vecs, c_vecs = [], []
    for t in range(T):
        f = float(2.0 ** t)
        s = pool.tile([128, w], F32, tag=f"bns{t}{sfx}", name=f"bns{t}{sfx}")
        cc = pool.tile([128, w], F32, tag=f"bnc{t}{sfx}", name=f"bnc{t}{sfx}")
        nc.vector.tensor_scalar(out=s, in0=scale, scalar1=f, scalar2=None,
                                op0=ALU.mult)
        nc.vector.tensor_scalar(out=cc, in0=shift, scalar1=f, scalar2=None,
                                op0=ALU.mult)
        s_vecs.append(s)
        c_vecs.append(cc)
    return s_vecs, c_vecs


def build_program(stop_after='D'):
    nc = bacc.Bacc("TRN2", target_bir_lowering=False, debug=False,
                   num_devices=N_CORES)

    xTr = nc.dram_tensor("xTr", [C, M], F32, kind="ExternalInput").ap()
    xTe = nc.dram_tensor("xTe", [C, M], BF16, kind="ExternalInput").ap()
    xTh = nc.dram_tensor("xTh", [C, M], BF16, kind="ExternalInput").ap()
    w1Tr = nc.dram_tensor("w1Tr", [C, H], F32, kind="ExternalInput").ap()
    w1Te = nc.dram_tensor("w1Te", [C, H], BF16, kind="ExternalInput").ap()
    w1Th = nc.dram_tensor("w1Th", [C, H], BF16, kind="ExternalInput").ap()
    w2Thi = nc.dram_tensor("w2Thi", [H, C], BF16, kind="ExternalInput").ap()
    w2Tlo = nc.dram_tensor("w2Tlo", [H, C], BF16, kind="ExternalInput").ap()
    g1 = nc.dram_tensor("g1", [H], F32, kind="ExternalInput").ap()
    b1 = nc.dram_tensor("b1", [H], F32, kind="ExternalInput").ap()
    g2 = nc.dram_tensor("g2", [C], F32, kind="ExternalInput").ap()
    b2 = nc.dram_tensor("b2", [C], F32, kind="ExternalInput").ap()
    qv = nc.dram_tensor("qv", [M, C], F32, kind="ExternalOutput").ap()
    chain = nc.dram_tensor("chain", [1, 128], F32, kind="ExternalInput").ap()
    chain_o = nc.dram_tensor("chain_o", [1, 128], F32, kind="ExternalOutput").ap()

    with tile.TileContext(nc) as tc:
        from contextlib import ExitStack
        with ExitStack() as ctx:
            _build_body(nc, tc, ctx, (xTr, xTe, xTh), (w1Tr, w1Te, w1Th),
                        w2Thi, w2Tlo, g1, b1, g2, b2, qv, stop_after)
        with tc.tile_pool(name="chainp", bufs=1) as chp:
            cht = chp.tile([1, 128], F32)
            nc.sync.dma_start(out=cht, in_=chain)
            nc.sync.dma_start(out=chain_o, in_=cht)
    nc.compile()
    return nc


def _build_body(nc, tc, ctx, x_in, w1_in, w2Thi, w2Tlo, g1, b1, g2, b2, qv,
                stop_after='D'):
    xTr, xTe, xTh = x_in
    w1Tr, w1Te, w1Th = w1_in
    F32R = mybir.dt.float32r
    HH = H_TILES // 2            # 8 h-tiles per half
    HCOL = HH * 128              # 1024 h columns per half

    persist = ctx.enter_context(tc.tile_pool(name="persist", bufs=1))
    dram = ctx.enter_context(tc.tile_pool(name="dram", bufs=1, space="DRAM"))

    id_bf = persist.tile([128, 128], BF16)
    make_identity(nc, id_bf)

    g1_sb = persist.tile([128, H_TILES], F32)
    b1_sb = persist.tile([128, H_TILES], F32)
    g2_sb = persist.tile([128, CT_TILES], F32)
    b2_sb = persist.tile([128, CT_TILES], F32)
    nc.sync.dma_start(out=g1_sb, in_=g1.rearrange("(a b) -> b a", b=128))
    nc.sync.dma_start(out=b1_sb, in_=b1.rearrange("(a b) -> b a", b=128))
    nc.sync.dma_start(out=g2_sb, in_=g2.rearrange("(a b) -> b a", b=128))
    nc.sync.dma_start(out=b2_sb, in_=b2.rearrange("(a b) -> b a", b=128))

    h_dram = dram.tile([H_TILES, 128, M], F32)
    spk_dram = dram.tile([M, H], BF16)
    ar1_in = dram.tile([2, 128, 2 * HH], F32)
    ar1_outs = [dram.tile([128, 2 * HH], F32, addr_space="Shared",
                          tag=f"ar1o{i}", name=f"ar1o{i}") for i in range(2)]
    ar2_in = dram.tile([2, 128, 4], F32)
    ar2_outs = [dram.tile([128, 4], F32, addr_space="Shared",
                          tag=f"ar2o{i}", name=f"ar2o{i}") for i in range(2)]

    # ================= A+B superphase: two ht-halves, interleaved =========
    with tc.tile_pool(name="pA_w", bufs=2) as paw, \
         tc.tile_pool(name="pA_wf", bufs=1) as pawf, \
         tc.tile_pool(name="pA_x", bufs=2) as pax, \
         tc.tile_pool(name="pA_h", bufs=4) as pah, \
         tc.tile_pool(name="pA_st", bufs=1) as past, \
         tc.tile_pool(name="pA_ps", bufs=5, space="PSUM") as paps, \
         tc.tile_pool(name="pB_h", bufs=10) as pbh, \
         tc.tile_pool(name="pB_v", bufs=3) as pbv, \
         tc.tile_pool(name="pB_sp", bufs=1) as pbsp, \
         tc.tile_pool(name="pB_stg", bufs=4) as pbstg, \
         tc.tile_pool(name="pB_ps", bufs=2, space="PSUM") as pbps:

        w1sbs = {}
        scales1 = {}
        sp_tiles = {}

        def emit_w1_loads(half):
            csl = slice(half * HCOL, (half + 1) * HCOL)
            w1r_sb, w1e_sb, w1h_sb = [], [], []
            for c in range(C_CHUNKS):
                wf = pawf.tile([128, HCOL], F32, tag="wf", name=f"wf{half}_{c}")
                nc.sync.dma_start(out=wf, in_=w1Tr[c * 128:(c + 1) * 128, csl])
                wr = paw.tile([128, HCOL], F32R, tag=f"w1r{c}",
                              name=f"w1r{half}_{c}")
                nc.vector.tensor_copy(wr, wf)
                w1r_sb.append(wr)
                we = paw.tile([128, HCOL], BF16, tag=f"w1e{c}",
                              name=f"w1e{half}_{c}")
                wh = paw.tile([128, HCOL], BF16, tag=f"w1h{c}",
                              name=f"w1h{half}_{c}")
                nc.sync.dma_start(out=we, in_=w1Te[c * 128:(c + 1) * 128, csl])
                nc.sync.dma_start(out=wh, in_=w1Th[c * 128:(c + 1) * 128, csl])
                w1e_sb.append(we)
                w1h_sb.append(wh)
            w1sbs[half] = (w1r_sb, w1e_sb, w1h_sb)

        st_tiles = {}

        def emit_A_mt(half, mt):
            w1r_sb, w1e_sb, w1h_sb = w1sbs[half]
            hts = list(range(half * HH, (half + 1) * HH))
            if mt == 0:
                st_tiles[half] = [past.tile([128, N_MT, 6], F32,
                                            tag=f"st{ht}", name=f"st{ht}")
                                  for ht in hts]
            st = st_tiles[half]
            msl = slice(mt * M_T, (mt + 1) * M_T)
            xr_sb, xe_sb, xh_sb = [], [], []
            for c in range(C_CHUNKS):
                xf = pax.tile([128, M_T], F32, tag=f"xf{c}",
                              name=f"xf{half}_{mt}_{c}")
                nc.sync.dma_start(out=xf, in_=xTr[c * 128:(c + 1) * 128, msl])
                xr = pax.tile([128, M_T], F32R, tag=f"xr{c}",
                              name=f"xr{half}_{mt}_{c}")
                nc.vector.tensor_copy(xr, xf)
                xr_sb.append(xr)
                xe = pax.tile([128, M_T], BF16, tag=f"xe{c}",
                              name=f"xe{half}_{mt}_{c}")
                xh = pax.tile([128, M_T], BF16, tag=f"xh{c}",
                              name=f"xh{half}_{mt}_{c}")
                nc.sync.dma_start(out=xe, in_=xTe[c * 128:(c + 1) * 128, msl])
                nc.sync.dma_start(out=xh, in_=xTh[c * 128:(c + 1) * 128, msl])
                xe_sb.append(xe)
                xh_sb.append(xh)
            for i_ht, ht in enumerate(hts):
                hsl = slice(i_ht * 128, (i_ht + 1) * 128)
                ps = paps.tile([128, M_T], F32, tag="ps")
                for c in range(C_CHUNKS):
                    nc.tensor.matmul(ps, w1r_sb[c][:, hsl], xr_sb[c],
                                     start=(c == 0), stop=False)
                for c in range(C_CHUNKS):
                    nc.tensor.matmul(ps, w1h_sb[c][:, hsl], xe_sb[c],
                                     start=False, stop=False)
                for c in range(C_CHUNKS):
                    nc.tensor.matmul(ps, w1e_sb[c][:, hsl], xh_sb[c],
                                     start=False, stop=(c == C_CHUNKS - 1))
                nc.vector.bn_stats(out=st[i_ht][:, mt, :], in_=ps)
                hstg = pah.tile([128, M_T], F32, tag="hstg")
                nc.scalar.activation(out=hstg, in_=ps, func=ACTF.Copy)
                nc.sync.dma_start(out=h_dram[ht][:, msl], in_=hstg)

        def emit_stats_ar1(half):
            hts = list(range(half * HH, (half + 1) * HH))
            st = st_tiles[half]
            stats1h = persist.tile([128, 2 * HH], F32, tag=f"stats1h{half}",
                                   name=f"stats1h{half}")
            for i_ht, ht in enumerate(hts):
                mv = past.tile([128, 2], F32, tag="mv", name=f"mv{ht}")
                nc.vector.bn_aggr(out=mv, in_=st[i_ht])
                nc.vector.tensor_copy(stats1h[:, i_ht:i_ht + 1], mv[:, 0:1])
                nc.vector.scalar_tensor_tensor(
                    out=stats1h[:, HH + i_ht:HH + i_ht + 1],
                    in0=mv[:, 0:1], scalar=mv[:, 0:1], in1=mv[:, 1:2],
                    op0=ALU.mult, op1=ALU.add)
            nc.sync.dma_start(out=ar1_in[half], in_=stats1h)
            nc.gpsimd.collective_compute(
                "AllReduce", ALU.add, replica_groups=[list(range(N_CORES))],
                ins=[ar1_in[half].opt()], outs=[ar1_outs[half].opt()])
            ar1h = persist.tile([128, 2 * HH], F32, tag=f"ar1h{half}",
                                name=f"ar1h{half}")
            nc.sync.dma_start(out=ar1h, in_=ar1_outs[half])
            scales1[half] = _emit_stats_to_scales(
                nc, persist, ar1h, g1_sb[:, half * HH:(half + 1) * HH],
                b1_sb[:, half * HH:(half + 1) * HH], HH, f"L1h{half}")

        def emit_B_tile(half, i_ht):
            s1v, c1v = scales1[half]
            ht = half * HH + i_ht
            hbs = []
            for tt in range(T):
                hbt = pbh.tile([128, NN * B_LOC], F32, tag="hb",
                               name=f"hb{ht}_{tt}")
                nc.sync.dma_start(
                    out=hbt,
                    in_=h_dram[ht][:, tt * NN * B_LOC:(tt + 1) * NN * B_LOC])
                hbs.append(hbt)
            sp = pbsp.tile([128, M], BF16, tag=f"sp{i_ht % 4}",
                           name=f"sp{half}_{i_ht}")
            _emit_lif(nc, hbs, sp, pbv,
                      [s[:, i_ht:i_ht + 1] for s in s1v],
                      [cv[:, i_ht:i_ht + 1] for cv in c1v],
                      NN * B_LOC)
            sp_tiles[(half, i_ht)] = sp

        def emit_B_trans(half, hg):
            sps = [sp_tiles[(half, hg * 4 + hh)] for hh in range(4)]
            hcol0 = (half * 2 + hg) * 512
            for mb in range(N_MB):
                pst = pbps.tile([MB, 512], BF16, tag="pst")
                for hh in range(4):
                    nc.tensor.matmul(
                        pst[:, hh * 128:(hh + 1) * 128],
                        sps[hh][:, mb * MB:(mb + 1) * MB],
                        id_bf, is_transpose=True,
                        start=(hh == 0), stop=(hh == 3))
                stg = pbstg.tile([MB, 512], BF16, tag="stg")
                if mb % 3 != 2:
                    nc.scalar.activation(out=stg, in_=pst, func=ACTF.Copy)
                else:
                    nc.vector.tensor_copy(stg, pst)
                nc.sync.dma_start(
                    out=spk_dram[mb * MB:(mb + 1) * MB, hcol0:hcol0 + 512],
                    in_=stg)

        # --- interleaved A/B emission ---
        emit_w1_loads(0)
        emit_w1_loads(1)
        for mt in range(N_MT):
            emit_A_mt(0, mt)
        emit_stats_ar1(0)
        for mt in range(N_MT):
            emit_A_mt(1, mt)
            emit_B_tile(0, mt)
            if mt == 3:
                emit_B_trans(0, 0)
        emit_stats_ar1(1)
        emit_B_trans(0, 1)
        for i_ht in range(HH):
            emit_B_tile(1, i_ht)
            if i_ht == 4:
                emit_B_trans(1, 0)
        emit_B_trans(1, 1)

    if stop_after in ('A', 'B'):
        return

    # ================= C+D superphase: two ct-halves, interleaved =========
    with tc.tile_pool(name="pC_w", bufs=1) as pcw, \
         tc.tile_pool(name="pC_r", bufs=2) as pcr, \
         tc.tile_pool(name="pC_st", bufs=1) as pcst, \
         tc.tile_pool(name="pC_ps", bufs=5, space="PSUM") as pcps, \
         tc.tile_pool(name="pO", bufs=1) as po, \
         tc.tile_pool(name="pD_v", bufs=2) as pdv, \
         tc.tile_pool(name="pD_sp", bufs=1) as pdsp, \
         tc.tile_pool(name="pD_stg", bufs=4) as pdstg, \
         tc.tile_pool(name="pD_ps", bufs=2, space="PSUM") as pdps:
        w2hi_sb = [pcw.tile([128, C], BF16, tag=f"w2h{i}", name=f"w2h{i}")
                   for i in range(H_TILES)]
        w2lo_sb = [pcw.tile([128, C], BF16, tag=f"w2l{i}", name=f"w2l{i}")
                   for i in range(H_TILES)]
        for i in range(H_TILES):
            nc.sync.dma_start(out=w2hi_sb[i], in_=w2Thi[i * 128:(i + 1) * 128, :])
            nc.sync.dma_start(out=w2lo_sb[i], in_=w2Tlo[i * 128:(i + 1) * 128, :])

        sp2 = [pdsp.tile([128, M], BF16, tag=f"sp2_{ct}", name=f"sp2_{ct}")
               for ct in range(CT_TILES)]
        o_tiles = {}
        st2_tiles = {}
        scales2 = {}

        def emit_C_mt(chalf, mt):
            cts = [2 * chalf, 2 * chalf + 1]
            if mt == 0:
                o_tiles[chalf] = [po.tile([128, M], F32, tag=f"osb{ct}",
                                          name=f"osb{ct}") for ct in cts]
                st2_tiles[chalf] = [pcst.tile([128, N_MT, 6], F32,
                                              tag=f"st2_{ct}",
                                              name=f"st2_{ct}") for ct in cts]
            o_sb = o_tiles[chalf]
            st2 = st2_tiles[chalf]
            rhs = []
            for ic in range(H_TILES):
                rt = pcr.tile([128, 2, NN], BF16, tag=f"rhs{ic}",
                              name=f"rhs{chalf}_{mt}_{ic}")
                src = bass.AP(
                    tensor=spk_dram.tensor,
                    offset=spk_dram.offset + (2 * mt) * (NN * H)
                    + (ic * 128) * NN,
                    ap=[[NN, 128], [NN * H, 2], [1, NN]])
                nc.sync.dma_start(out=rt, in_=src)
                rhs.append(rt)
            for i_ct, ct in enumerate(cts):
                ps = pcps.tile([128, M_T], F32, tag="ps2")
                for ic in range(H_TILES):
                    nc.tensor.matmul(
                        ps, w2hi_sb[ic][:, ct * 128:(ct + 1) * 128],
                        rhs[ic].rearrange("p a b -> p (a b)"),
                        start=(ic == 0), stop=False)
                for ic in range(H_TILES):
                    nc.tensor.matmul(
                        ps, w2lo_sb[ic][:, ct * 128:(ct + 1) * 128],
                        rhs[ic].rearrange("p a b -> p (a b)"),
                        start=False, stop=(ic == H_TILES - 1))
                nc.vector.bn_stats(out=st2[i_ct][:, mt, :], in_=ps)
                nc.scalar.activation(
                    out=o_sb[i_ct][:, mt * M_T:(mt + 1) * M_T],
                    in_=ps, func=ACTF.Copy)

        def emit_stats_ar2(chalf):
            cts = [2 * chalf, 2 * chalf + 1]
            st2 = st2_tiles[chalf]
            stats2h = persist.tile([128, 4], F32, tag=f"stats2h{chalf}",
                                   name=f"stats2h{chalf}")
            for i_ct, ct in enumerate(cts):
                mv2 = pcst.tile([128, 2], F32, tag="mv2", name=f"mv2{ct}")
                nc.vector.bn_aggr(out=mv2, in_=st2[i_ct])
                nc.vector.tensor_copy(stats2h[:, i_ct:i_ct + 1], mv2[:, 0:1])
                nc.vector.scalar_tensor_tensor(
                    out=stats2h[:, 2 + i_ct:2 + i_ct + 1],
                    in0=mv2[:, 0:1], scalar=mv2[:, 0:1], in1=mv2[:, 1:2],
                    op0=ALU.mult, op1=ALU.add)
            nc.sync.dma_start(out=ar2_in[chalf], in_=stats2h)
            nc.gpsimd.collective_compute(
                "AllReduce", ALU.add, replica_groups=[list(range(N_CORES))],
                ins=[ar2_in[chalf].opt()], outs=[ar2_outs[chalf].opt()])
            ar2h = persist.tile([128, 4], F32, tag=f"ar2h{chalf}",
                                name=f"ar2h{chalf}")
            nc.sync.dma_start(out=ar2h, in_=ar2_outs[chalf])
            scales2[chalf] = _emit_stats_to_scales(
                nc, persist, ar2h, g2_sb[:, 2 * chalf:2 * chalf + 2],
                b2_sb[:, 2 * chalf:2 * chalf + 2], 2, f"L2h{chalf}")

        def emit_D_lif(chalf, i_ct):
            s2v, c2v = scales2[chalf]
            ct = 2 * chalf + i_ct
            _emit_lif(nc, o_tiles[chalf][i_ct], sp2[ct], pdv,
                      [s[:, i_ct:i_ct + 1] for s in s2v],
                      [cv[:, i_ct:i_ct + 1] for cv in c2v],
                      NN * B_LOC)

        def emit_D_trans(chalf):
            cts = [2 * chalf, 2 * chalf + 1]
            for mb in range(N_MB):
                pst = pdps.tile([MB, 256], BF16, tag="pst2")
                for i_ct, ct in enumerate(cts):
                    nc.tensor.matmul(
                        pst[:, i_ct * 128:(i_ct + 1) * 128],
                        sp2[ct][:, mb * MB:(mb + 1) * MB],
                        id_bf, is_transpose=True,
                        start=(i_ct == 0), stop=(i_ct == 1))
                stg = pdstg.tile([MB, 256], F32, tag="stg2")
                if mb % 2 == 0:
                    nc.scalar.activation(out=stg, in_=pst, func=ACTF.Copy)
                else:
                    nc.vector.tensor_copy(stg, pst)
                nc.sync.dma_start(
                    out=qv[mb * MB:(mb + 1) * MB,
                           chalf * 256:(chalf + 1) * 256],
                    in_=stg)

        # --- interleaved C/D emission ---
        for mt in range(N_MT):
            emit_C_mt(0, mt)
        emit_stats_ar2(0)
        for mt in range(N_MT):
            emit_C_mt(1, mt)
            if mt == 1:
                emit_D_lif(0, 0)
            elif mt == 3:
                emit_D_lif(0, 1)
        emit_stats_ar2(1)
        emit_D_trans(0)
        emit_D_lif(1, 0)
        emit_D_lif(1, 1)
        emit_D_trans(1)


_NC_CACHE = None
LAST_RES = None


def _get_nc():
    global _NC_CACHE
    if _NC_CACHE is None:
        _NC_CACHE = build_program()
    return _NC_CACHE


def kernel(x, w1, g1, b1, w2, g2, b2):
    x = np.asarray(x, dtype=np.float32)
    w1 = np.asarray(w1, dtype=np.float32)
    w2 = np.asarray(w2, dtype=np.float32)
    g1 = np.asarray(g1, dtype=np.float32)
    b1 = np.asarray(b1, dtype=np.float32)
    g2 = np.asarray(g2, dtype=np.float32)
    b2 = np.asarray(b2, dtype=np.float32)

    w1T = np.ascontiguousarray(w1.T)                    # (C, H)
    w2T = np.ascontiguousarray(w2.T).astype(np.float32)  # (H, C)
    w2Thi = w2T.astype(ml_dtypes.bfloat16)
    w2Tlo = (w2T - w2Thi.astype(np.float32)).astype(ml_dtypes.bfloat16)
    w1Tr = rne_keep(w1T, 11)
    w1Te = (w1T - w1Tr).astype(ml_dtypes.bfloat16)
    w1Th = w1T.astype(ml_dtypes.bfloat16)

    xr = x.reshape(T, B_GLOB, NN, C)
    in_maps = []
    for k in range(N_CORES):
        xk = xr[:, k * B_LOC:(k + 1) * B_LOC].reshape(M, C)
        xTk = np.ascontiguousarray(xk.T)                # (C, M)
        xTr = rne_keep(xTk, 11)
        in_maps.append({
            "xTr": xTr,
            "xTe": (xTk - xTr).astype(ml_dtypes.bfloat16),
            "xTh": xTk.astype(ml_dtypes.bfloat16),
            "w1Tr": w1Tr, "w1Te": w1Te, "w1Th": w1Th,
            "w2Thi": w2Thi, "w2Tlo": w2Tlo,
            "g1": g1, "b1": b1, "g2": g2, "b2": b2,
            "chain": np.zeros((1, 128), np.float32),
        })

    nc = _get_nc()
    import os
    trace = bool(int(os.environ.get("KERNEL_TRACE", "0")))
    res = run_bass_kernel_spmd(nc, in_maps, core_ids=list(range(N_CORES)),
                               trace=trace)
    global LAST_RES
    LAST_RES = res

    out = np.empty((T * B_GLOB, NN, C), dtype=np.float32)
    outr = out.reshape(T, B_GLOB, NN, C)
    for k in range(N_CORES):
        qvk = res.results[k]["qv"]                      # (M, C) in (r, n, c)
        # reference: reshape(TB, C, N).transpose(0, 2, 1)
        tmp = qvk.reshape(R, C, NN).transpose(0, 2, 1)  # (R, N, C)
        outr[:, k * B_LOC:(k + 1) * B_LOC] = tmp.reshape(T, B_LOC, NN, C)
    return out



# revision 36
# speedup vs baseline: 1.2625x; 1.2625x over previous
"""Trainium2 Bass kernel for nn_Decoder_MLP: Linear->BN->LIF -> Linear->BN->LIF.

Sharding: data-parallel over batch B (TB=T*B=128 rows -> 4 batch items/core,
all T=4 timesteps local). BN batch stats are all-reduced across the 8 cores
(8 group-ARs for layer 1, 2 pair-ARs for layer 2), each hidden under compute.

v2 design (cost-model driven):
- GEMM1 (x@w1T): 3 passes -- fp16(x)@fp16(w1) main + bf16(dx)@fp16(w1) +
  fp16(x)@bf16(dw) crosses. fp16 == RNE-11, so precision ~2^-21. h stays in
  SBUF (no DRAM roundtrip); BN stats all-reduced per 2-ht group so LIF of
  group g runs under the matmuls of group g+1.
- Spikes are exact in fp8e4; stored to DRAM as [m, h] fp8 (the reference's
  "scrambled" reshape is a flat reinterpretation within each r-block, so the
  scrambled read back is an affine AP).
- GEMM2 (s@w2T): w2 split into 4 e4m3 terms (scales 2^5,2^9,2^13,2^17);
  pairs of K-chunks packed into fp8 DoubleRow matmuls (0.5 cyc/row) ->
  same cost as ONE bf16 pass. Terms combine via 2 gpsimd + 1 DVE op per
  tile; the 2^5 overall scale is absorbed exactly by BN (eps scaled 2^10).
"""

import numpy as np
import ml_dtypes

import concourse.bass as bass
import concourse.mybir as mybir
import concourse.tile as tile
from concourse import bacc
from concourse.bass_utils import run_bass_kernel_spmd
from concourse.masks import make_identity

F32 = mybir.dt.float32
F16 = mybir.dt.float16
BF16 = mybir.dt.bfloat16
F8 = mybir.dt.float8e4
ALU = mybir.AluOpType
ACTF = mybir.ActivationFunctionType
DR = mybir.MatmulPerfMode.DoubleRow

N_CORES = 8
T = 4
B_GLOB = 32
B_LOC = B_GLOB // N_CORES          # 4 batch items per core
R = T * B_LOC                      # 16 local (t, b) rows
NN = 196
C = 512
H = 2048
M = R * NN                         # 3136 local rows
M_T = 392                          # m-tile (2 r-rows)
N_MT = M // M_T                    # 8
MB = 112                           # transpose block (3136 = 28*112)
N_MB = M // MB                     # 28
NG = 8                             # ht-groups for layer 1 (2 ht each)
HPG = 2                            # ht per group
EPS = 1e-5
EPS2 = 1e-5 * 1024.0               # layer-2 stats are of o' = 2^5 o
W2S = [2.0 ** 5, 2.0 ** 9, 2.0 ** 13, 2.0 ** 17]


def _emit_lif(nc, lanes, vpool, n_free, n_t=T, n_chunk=1, vp_on_pool=False):
    """BN-apply + LIF, multi-lane and m-chunked so independent dependency
    chains pipeline across engines. lanes: list of (sb_h [128, n_t*n_free],
    sp_out, s_vecs, c_vecs). W_t = W'_{t-1} + 2^t*(scale*h_t + shift);
    spike iff W_t >= 2^{t+1}; W'_t = W_t*(W_t < 2^{t+1})."""
    nf = n_free // n_chunk
    vprev = {}
    for t in range(n_t):
        thr = float(2.0 ** (t + 1))
        for li, (sb_h, sp_out, s_vecs, c_vecs) in enumerate(lanes):
            for ck in range(n_chunk):
                sl = slice(t * n_free + ck * nf, t * n_free + (ck + 1) * nf)
                key = (li, ck)
                if t == 0:
                    v = vpool.tile([128, nf], F32, tag=f"v{li}{ck}")
                    nc.scalar.activation(out=v, in_=sb_h[:, sl],
                                         func=ACTF.Identity,
                                         bias=c_vecs[t], scale=s_vecs[t])
                else:
                    y = vpool.tile([128, nf], F32, tag=f"y{li}{ck}")
                    nc.scalar.activation(out=y, in_=sb_h[:, sl],
                                         func=ACTF.Identity,
                                         bias=c_vecs[t], scale=s_vecs[t])
                    v = vpool.tile([128, nf], F32, tag=f"v{li}{ck}")
                    nc.vector.tensor_tensor(out=v, in0=vprev[key], in1=y,
                                            op=ALU.add)
                nc.gpsimd.tensor_scalar(out=sp_out[:, sl], in0=v,
                                        scalar1=thr, scalar2=None,
                                        op0=ALU.is_ge)
                if t < n_t - 1:
                    vp = vpool.tile([128, nf], F32, tag=f"vp{li}{ck}")
                    nc.vector.scalar_tensor_tensor(out=vp, in0=v, scalar=thr,
                                                   in1=v, op0=ALU.is_lt,
                                                   op1=ALU.mult)
                    vprev[key] = vp


def _emit_stats_to_scales(nc, pool, ar_sb, gamma_sb, beta_sb, w, sfx, eps):
    """ar_sb: (128, 2w) all-reduced [sums-of-means | sums-of-Ex2]. Returns
    per-t (s_vecs, c_vecs) lists of (128, w) tiles: 2^t*scale, 2^t*shift."""
    mean = pool.tile([128, w], F32, tag=f"bnmean{sfx}", name=f"bnmean{sfx}")
    ex2 = pool.tile([128, w], F32, tag=f"bnex2{sfx}", name=f"bnex2{sfx}")
    nc.vector.tensor_scalar(out=mean, in0=ar_sb[:, 0:w], scalar1=1.0 / N_CORES,
                            scalar2=None, op0=ALU.mult)
    nc.vector.tensor_scalar(out=ex2, in0=ar_sb[:, w:2 * w],
                            scalar1=1.0 / N_CORES, scalar2=None, op0=ALU.mult)
    var = pool.tile([128, w], F32, tag=f"bnvar{sfx}", name=f"bnvar{sfx}")
    msq = pool.tile([128, w], F32, tag=f"bnmsq{sfx}", name=f"bnmsq{sfx}")
    nc.vector.tensor_tensor(out=msq, in0=mean, in1=mean, op=ALU.mult)
    nc.vector.tensor_tensor(out=var, in0=ex2, in1=msq, op=ALU.subtract)
    epsb = pool.tile([128, 1], F32, tag=f"bneps{sfx}", name=f"bneps{sfx}")
    nc.vector.memset(epsb, eps)
    std = pool.tile([128, w], F32, tag=f"bnstd{sfx}", name=f"bnstd{sfx}")
    nc.scalar.activation(out=std, in_=var, func=ACTF.Sqrt, bias=epsb, scale=1.0)
    rstd = pool.tile([128, w], F32, tag=f"bnrstd{sfx}", name=f"bnrstd{sfx}")
    nc.vector.reciprocal(out=rstd, in_=std)
    scale = pool.tile([128, w], F32, tag=f"bnscale{sfx}", name=f"bnscale{sfx}")
    nc.vector.tensor_tensor(out=scale, in0=gamma_sb, in1=rstd, op=ALU.mult)
    mscl = pool.tile([128, w], F32, tag=f"bnmscl{sfx}", name=f"bnmscl{sfx}")
    nc.vector.tensor_tensor(out=mscl, in0=mean, in1=scale, op=ALU.mult)
    shift = pool.tile([128, w], F32, tag=f"bnshift{sfx}", name=f"bnshift{sfx}")
    nc.vector.tensor_tensor(out=shift, in0=beta_sb, in1=mscl, op=ALU.subtract)
    s_vecs, c_vecs = [], []
    for t in range(T):
        f = float(2.0 ** t)
        s = pool.tile([128, w], F32, tag=f"bns{t}{sfx}", name=f"bns{t}{sfx}")
        cc = pool.tile([128, w], F32, tag=f"bnc{t}{sfx}", name=f"bnc{t}{sfx}")
        nc.vector.tensor_scalar(out=s, in0=scale, scalar1=f, scalar2=None,
                                op0=ALU.mult)
        nc.vector.tensor_scalar(out=cc, in0=shift, scalar1=f, scalar2=None,
                                op0=ALU.mult)
        s_vecs.append(s)
        c_vecs.append(cc)
    return s_vecs, c_vecs


def build_program():
    nc = bacc.Bacc("TRN2", target_bir_lowering=False, debug=False,
                   num_devices=N_CORES)

    xT16 = nc.dram_tensor("xT16", [C, M], F16, kind="ExternalInput").ap()
    dxTb = nc.dram_tensor("dxTb", [C, M], BF16, kind="ExternalInput").ap()
    w1T16 = nc.dram_tensor("w1T16", [C, H], F16, kind="ExternalInput").ap()
    dw1Tb = nc.dram_tensor("dw1Tb", [C, H], BF16, kind="ExternalInput").ap()
    w2p = nc.dram_tensor("w2p", [4 * 8 * 128, 2 * C], F8,
                         kind="ExternalInput").ap()
    g1 = nc.dram_tensor("g1", [H], F32, kind="ExternalInput").ap()
    b1 = nc.dram_tensor("b1", [H], F32, kind="ExternalInput").ap()
    g2 = nc.dram_tensor("g2", [C], F32, kind="ExternalInput").ap()
    b2 = nc.dram_tensor("b2", [C], F32, kind="ExternalInput").ap()
    qv = nc.dram_tensor("qv", [M, C], BF16, kind="ExternalOutput").ap()
    chain = nc.dram_tensor("chain", [1, 128], F32, kind="ExternalInput").ap()
    chain_o = nc.dram_tensor("chain_o", [1, 128], F32, kind="ExternalOutput").ap()

    with tile.TileContext(nc) as tc:
        _build_body(nc, tc, xT16, dxTb, w1T16, dw1Tb, w2p, g1, b1, g2, b2, qv)
        with tc.tile_pool(name="chainp", bufs=1) as chp:
            cht = chp.tile([1, 128], F32)
            nc.sync.dma_start(out=cht, in_=chain)
            nc.sync.dma_start(out=chain_o, in_=cht)
    nc.compile()
    return nc


def _build_body(nc, tc, xT16, dxTb, w1T16, dw1Tb, w2p, g1, b1, g2, b2, qv):
    from contextlib import ExitStack

    with ExitStack() as octx:
        persist = octx.enter_context(tc.tile_pool(name="persist", bufs=1))
        dram = octx.enter_context(tc.tile_pool(name="dram", bufs=1, space="DRAM"))

        id_bf = persist.tile([128, 128], BF16)
        g1_sb = persist.tile([128, 16], F32)
        b1_sb = persist.tile([128, 16], F32)
        g2_sb = persist.tile([128, 4], F32)
        b2_sb = persist.tile([128, 4], F32)

        def emit_globals():
            # deferred: not needed until B(0)/trans; keeps the SP queue
            # clear for the first w1/x loads
            make_identity(nc, id_bf)
            nc.sync.dma_start(out=g1_sb, in_=g1.rearrange("(a b) -> b a", b=128))
            nc.sync.dma_start(out=b1_sb, in_=b1.rearrange("(a b) -> b a", b=128))
            nc.sync.dma_start(out=g2_sb, in_=g2.rearrange("(a b) -> b a", b=128))
            nc.sync.dma_start(out=b2_sb, in_=b2.rearrange("(a b) -> b a", b=128))

        spk = dram.tile([M, H], F8)
        ar1_in = dram.tile([NG, 128, 4], F32)
        ar1_outs = [dram.tile([128, 4], F32, addr_space="Shared",
                              tag=f"ar1o{g}", name=f"ar1o{g}") for g in range(NG)]
        ar2_in = dram.tile([2, 128, 4], F32)
        ar2_outs = [dram.tile([128, 4], F32, addr_space="Shared",
                              tag=f"ar2o{i}", name=f"ar2o{i}") for i in range(2)]

        scales1 = {}
        scales2 = {}
        sp_tiles = {}
        st_tiles = {}
        w1sbs = {}
        h_tiles = {}

        # ============== A+B superphase: 8 ht-groups pipelined ==============
        with tc.tile_pool(name="px", bufs=1) as px, \
             tc.tile_pool(name="pw1", bufs=2) as pw1, \
             tc.tile_pool(name="ph", bufs=2) as ph, \
             tc.tile_pool(name="pA_st", bufs=1) as past, \
             tc.tile_pool(name="pA_ps", bufs=6, space="PSUM") as paps, \
             tc.tile_pool(name="pB_v", bufs=3) as pbv, \
             tc.tile_pool(name="pB_sp", bufs=2) as pbsp, \
             tc.tile_pool(name="pB_stg", bufs=2) as pbstg, \
             tc.tile_pool(name="pB_ps", bufs=2, space="PSUM") as pbps:

            x16_lo = px.tile([128, 2, M], F16)
            x16_hi = px.tile([128, 2, M], F16)
            dxb_sb = px.tile([128, 4, M], BF16)

            def x16_sl(c, msl):
                return (x16_lo if c < 2 else x16_hi)[:, c % 2, msl]

            def emit_x_load(part):           # 4 parts of 2 mt each
                msl = slice(part * 784, (part + 1) * 784)
                for half, xt in ((0, x16_lo), (1, x16_hi)):
                    nc.sync.dma_start(
                        out=xt[:, :, msl],
                        in_=bass.AP(tensor=xT16.tensor,
                                    offset=xT16.offset + part * 784
                                    + half * 2 * 128 * M,
                                    ap=[[M, 128], [128 * M, 2], [1, 784]]))
                nc.sync.dma_start(
                    out=dxb_sb[:, :, msl],
                    in_=bass.AP(tensor=dxTb.tensor,
                                offset=dxTb.offset + part * 784,
                                ap=[[M, 128], [128 * M, 4], [1, 784]]))

            def emit_w1_load(g):
                csl = slice(g * HPG * 128, (g + 1) * HPG * 128)
                w16 = pw1.tile([128, 4, HPG * 128], F16, tag="w16",
                               name=f"w16_{g}")
                wbf = pw1.tile([128, 4, HPG * 128], BF16, tag="wbf",
                               name=f"wbf_{g}")
                nc.sync.dma_start(
                    out=w16,
                    in_=bass.AP(tensor=w1T16.tensor,
                                offset=w1T16.offset + g * HPG * 128,
                                ap=[[H, 128], [128 * H, 4], [1, HPG * 128]]))
                nc.sync.dma_start(
                    out=wbf,
                    in_=bass.AP(tensor=dw1Tb.tensor,
                                offset=dw1Tb.offset + g * HPG * 128,
                                ap=[[H, 128], [128 * H, 4], [1, HPG * 128]]))
                w1sbs[g] = (w16, wbf)

            def emit_A_group(g):
                w16, wbf = w1sbs[g]
                hts = [g * HPG + i for i in range(HPG)]
                for ht in hts:
                    st_tiles[ht] = past.tile([128, N_MT, 6], F32,
                                             tag=f"st{ht}", name=f"st{ht}")
                    h_tiles[ht] = ph.tile([128, M], F32, tag=f"h{ht % HPG}",
                                          name=f"h_{ht}")
                for mt in range(N_MT):
                    msl = slice(mt * M_T, (mt + 1) * M_T)
                    for hl, ht in enumerate(hts):
                        hsl = slice(hl * 128, (hl + 1) * 128)
                        ps = paps.tile([128, M_T], F32, tag="ps")
                        for c in range(4):
                            nc.tensor.matmul(ps, w16[:, c, hsl],
                                             x16_sl(c, msl),
                                             start=(c == 0), stop=False)
                        for c in range(4):
                            nc.tensor.matmul(ps, w16[:, c, hsl],
                                             dxb_sb[:, c, msl],
                                             start=False, stop=False)
                        for c in range(4):
                            nc.tensor.matmul(ps, wbf[:, c, hsl],
                                             x16_sl(c, msl),
                                             start=False, stop=(c == 3))
                        # alternate drain engine so one engine's LIF burst
                        # never stalls PSUM recycling
                        if mt % 2 == 0:
                            nc.scalar.activation(out=h_tiles[ht][:, msl],
                                                 in_=ps, func=ACTF.Copy)
                        else:
                            nc.vector.tensor_copy(h_tiles[ht][:, msl], ps)
                        nc.vector.bn_stats(out=st_tiles[ht][:, mt, :],
                                           in_=h_tiles[ht][:, msl])

            def emit_stats_out(g):
                hts = [g * HPG + i for i in range(HPG)]
                stats = persist.tile([128, 4], F32, tag=f"stats1_{g}",
                                     name=f"stats1_{g}")
                for hl, ht in enumerate(hts):
                    mv = past.tile([128, 2], F32, tag="mv", name=f"mv{ht}")
                    nc.vector.bn_aggr(out=mv, in_=st_tiles[ht])
                    nc.vector.tensor_copy(stats[:, hl:hl + 1], mv[:, 0:1])
                    nc.vector.scalar_tensor_tensor(
                        out=stats[:, HPG + hl:HPG + hl + 1],
                        in0=mv[:, 0:1], scalar=mv[:, 0:1], in1=mv[:, 1:2],
                        op0=ALU.mult, op1=ALU.add)
                nc.sync.dma_start(out=ar1_in[g], in_=stats)
                nc.gpsimd.collective_compute(
                    "AllReduce", ALU.add, replica_groups=[list(range(N_CORES))],
                    ins=[ar1_in[g].opt()], outs=[ar1_outs[g].opt()])

            def emit_scales1(g):
                ar = persist.tile([128, 4], F32, tag=f"ar1_{g}",
                                  name=f"ar1_{g}")
                nc.sync.dma_start(out=ar, in_=ar1_outs[g])
                scales1[g] = _emit_stats_to_scales(
                    nc, persist, ar,
                    g1_sb[:, g * HPG:(g + 1) * HPG],
                    b1_sb[:, g * HPG:(g + 1) * HPG], HPG, f"L1g{g}", EPS)

            def emit_B_group(g):
                emit_scales1(g)
                s1v, c1v = scales1[g]
                for hl in range(HPG):
                    ht = g * HPG + hl
                    sp = pbsp.tile([128, M], BF16, tag=f"s{ht % 4}",
                                   name=f"sp_{ht}")
                    _emit_lif(nc, [(h_tiles[ht], sp,
                                    [s[:, hl:hl + 1] for s in s1v],
                                    [cv[:, hl:hl + 1] for cv in c1v])],
                              pbv, NN * B_LOC)
                    sp_tiles[ht] = sp

            def emit_B_trans(hts, hcol0):
                """Transpose spikes of `hts` (columns hcol0..+128*len) into
                spk DRAM; len(hts) in {2, 4}."""
                nh = len(hts)
                w = nh * 128
                sps = [sp_tiles[ht] for ht in hts]
                stg = None
                for mb in range(N_MB):
                    pstf = pbps.tile([MB, 512], BF16, tag="pst")
                    pst = pstf[:, 0:w] if w < 512 else pstf
                    for hh in range(nh):
                        nc.tensor.matmul(
                            pst[:, hh * 128:(hh + 1) * 128],
                            sps[hh][:, mb * MB:(mb + 1) * MB],
                            id_bf, is_transpose=True,
                            start=(hh == 0), stop=(hh == nh - 1))
                    if mb % 7 == 0:
                        stg = pbstg.tile([MB, 7, w], F8, tag=f"stg{w}")
                    if mb % 2 == 0:
                        nc.scalar.activation(out=stg[:, mb % 7, :], in_=pst,
                                             func=ACTF.Copy)
                    else:
                        nc.vector.tensor_copy(stg[:, mb % 7, :], pst)
                    if mb % 7 == 6:
                        m0 = (mb - 6) * MB
                        nc.sync.dma_start(
                            out=bass.AP(tensor=spk.tensor,
                                        offset=spk.offset + m0 * H + hcol0,
                                        ap=[[H, MB], [MB * H, 7], [1, w]]),
                            in_=stg)

            # ---- A/B emission schedule ----
            emit_w1_load(0)
            emit_x_load(0)
            emit_w1_load(1)
            emit_globals()
            for part in range(1, 4):
                emit_x_load(part)
            for g in range(NG):
                emit_A_group(g)
                emit_stats_out(g)
                if g + 1 < NG:
                    emit_w1_load(g + 1)
                if g >= 1:
                    emit_B_group(g - 1)       # LIF of g-1 under A(g+1) emission
                if g == 3:
                    emit_B_trans([0, 1, 2, 3], 0)
                if g == 5:
                    emit_B_trans([4, 5, 6, 7], 512)
            emit_B_group(NG - 1)
            emit_B_trans([8, 9, 10, 11], 1024)      # AR(7)-window fill
            emit_B_trans([12, 13], 1536)
            emit_B_trans([14, 15], 1792)            # gated on B(7) LIF

        # ============== C+D superphase ==============
        with tc.tile_pool(name="pw2", bufs=1) as pw2, \
             tc.tile_pool(name="prhs", bufs=1) as prhs, \
             tc.tile_pool(name="po", bufs=2) as po, \
             tc.tile_pool(name="pC_st", bufs=1) as pcst, \
             tc.tile_pool(name="pC_tmp", bufs=2) as ptmp, \
             tc.tile_pool(name="pC_ps23", bufs=1, space="PSUM") as pp23, \
             tc.tile_pool(name="pC_ps01", bufs=2, space="PSUM") as pp01, \
             tc.tile_pool(name="pD_v", bufs=1) as pdv, \
             tc.tile_pool(name="pD_sp", bufs=2) as pdsp, \
             tc.tile_pool(name="pD_stg", bufs=2) as pdstg, \
             tc.tile_pool(name="pD_ps", bufs=2, space="PSUM") as pdps:

            w2sb = []
            for k in range(4):
                wt = pw2.tile([128, 8, 2, C], F8, tag=f"w2_{k}",
                              name=f"w2_{k}")
                nc.sync.dma_start(
                    out=wt,
                    in_=bass.AP(tensor=w2p.tensor,
                                offset=w2p.offset + k * 8 * 128 * 2 * C,
                                ap=[[2 * C, 128], [128 * 2 * C, 8], [1, 2 * C]]))
                w2sb.append(wt)

            rhs = [[None, None] for _ in range(8)]   # rhs[ip][r-half]
            for hf in range(2):
                for ip in range(8):
                    rt = prhs.tile([128, 2, 8, NN], F8, tag=f"rhs{ip}_{hf}",
                                   name=f"rhs{ip}_{hf}")
                    for pl in range(2):
                        nc.sync.dma_start(
                            out=rt[:, pl, :, :],
                            in_=bass.AP(tensor=spk.tensor,
                                        offset=spk.offset
                                        + (ip * 2 + pl) * 128 * NN
                                        + hf * 8 * NN * H,
                                        ap=[[NN, 128], [NN * H, 8], [1, NN]]))
                    rhs[ip][hf] = rt

            o_tiles = {}
            st2_tiles = {}
            sp2 = {}

            def emit_C_ct(ct):
                o_tiles[ct] = po.tile([128, M], F32, tag=f"o{ct % 2}",
                                      name=f"o_{ct}")
                st2_tiles[ct] = pcst.tile([128, N_MT, 6], F32,
                                          tag=f"st2_{ct}", name=f"st2_{ct}")
                csl = slice(ct * 128, (ct + 1) * 128)
                for mt in range(N_MT):
                    msl = slice(mt * M_T, (mt + 1) * M_T)
                    pss = {}
                    for k in (3, 2, 1, 0):
                        pool = pp23 if k >= 2 else pp01
                        ps = pool.tile([128, M_T], F32, tag=f"p{k}")
                        for ip in range(8):
                            rr = rhs[ip][mt // 4][:, :, 2 * (mt % 4):
                                                  2 * (mt % 4) + 2, :] \
                                .rearrange("p a b c -> p a (b c)")
                            nc.tensor.matmul(ps, w2sb[k][:, ip, :, csl], rr,
                                             start=(ip == 0), stop=(ip == 7),
                                             perf_mode=DR)
                        pss[k] = ps
                        if k == 3:
                            # ACT prescale: only one PSUM input allowed per op
                            c3 = ptmp.tile([128, M_T], F32, tag="c3")
                            nc.scalar.activation(out=c3, in_=pss[3],
                                                 func=ACTF.Copy,
                                                 scale=2.0 ** -4)
                        if k == 2:
                            t1 = ptmp.tile([128, M_T], F32, tag="t1")
                            nc.vector.tensor_tensor(out=t1, in0=c3,
                                                    in1=pss[2], op=ALU.add)
                    t2 = ptmp.tile([128, M_T], F32, tag="t2")
                    nc.vector.scalar_tensor_tensor(
                        out=t2, in0=t1, scalar=2.0 ** -4, in1=pss[1],
                        op0=ALU.mult, op1=ALU.add)
                    osl = o_tiles[ct][:, msl]
                    nc.vector.scalar_tensor_tensor(
                        out=osl, in0=t2, scalar=2.0 ** -4, in1=pss[0],
                        op0=ALU.mult, op1=ALU.add)
                    nc.vector.bn_stats(out=st2_tiles[ct][:, mt, :], in_=osl)

            def emit_stats2_out(pair):
                cts = [2 * pair, 2 * pair + 1]
                stats = persist.tile([128, 4], F32, tag=f"stats2_{pair}",
                                     name=f"stats2_{pair}")
                for i, ct in enumerate(cts):
                    mv2 = pcst.tile([128, 2], F32, tag="mv2", name=f"mv2{ct}")
                    nc.vector.bn_aggr(out=mv2, in_=st2_tiles[ct])
                    nc.vector.tensor_copy(stats[:, i:i + 1], mv2[:, 0:1])
                    nc.vector.scalar_tensor_tensor(
                        out=stats[:, 2 + i:2 + i + 1],
                        in0=mv2[:, 0:1], scalar=mv2[:, 0:1], in1=mv2[:, 1:2],
                        op0=ALU.mult, op1=ALU.add)
                nc.sync.dma_start(out=ar2_in[pair], in_=stats)
                nc.gpsimd.collective_compute(
                    "AllReduce", ALU.add, replica_groups=[list(range(N_CORES))],
                    ins=[ar2_in[pair].opt()], outs=[ar2_outs[pair].opt()])

            def emit_D_pair(pair):
                # allocate from the C-combine pool: the slot-reuse WAR edge
                # stops the scheduler hoisting this AR-gated chain into the
                # middle of C (DVE/Pool head-of-line blocking)
                ar = ptmp.tile([128, 4], F32, tag="t2", name=f"ar2_{pair}")
                nc.sync.dma_start(out=ar, in_=ar2_outs[pair])
                scales2[pair] = _emit_stats_to_scales(
                    nc, persist, ar, g2_sb[:, 2 * pair:2 * pair + 2],
                    b2_sb[:, 2 * pair:2 * pair + 2], 2, f"L2p{pair}", EPS2)
                s2v, c2v = scales2[pair]
                lanes = []
                for i in range(2):
                    ct = 2 * pair + i
                    sp = pdsp.tile([128, M], BF16, tag=f"sp2_{i}",
                                   name=f"sp2_{ct}")
                    lanes.append((o_tiles[ct], sp,
                                  [s[:, i:i + 1] for s in s2v],
                                  [cv[:, i:i + 1] for cv in c2v]))
                    sp2[ct] = sp
                _emit_lif(nc, lanes, pdv, NN * B_LOC, n_chunk=2)

            def emit_D_trans(pair):
                cts = [2 * pair, 2 * pair + 1]
                stg = None
                for mb in range(N_MB):
                    pst = pdps.tile([MB, 256], BF16, tag="pst2")
                    for i, ct in enumerate(cts):
                        nc.tensor.matmul(
                            pst[:, i * 128:(i + 1) * 128],
                            sp2[ct][:, mb * MB:(mb + 1) * MB],
                            id_bf, is_transpose=True,
                            start=(i == 0), stop=(i == 1))
                    if mb % 7 == 0:
                        stg = pdstg.tile([MB, 7, 256], BF16, tag="stg2")
                    if mb % 2 == 0:
                        nc.scalar.activation(out=stg[:, mb % 7, :], in_=pst,
                                             func=ACTF.Copy)
                    else:
                        nc.vector.tensor_copy(stg[:, mb % 7, :], pst)
                    if mb % 7 == 6:
                        m0 = (mb - 6) * MB
                        nc.sync.dma_start(
                            out=bass.AP(tensor=qv.tensor,
                                        offset=qv.offset + m0 * C + pair * 256,
                                        ap=[[C, MB], [MB * C, 7], [1, 256]]),
                            in_=stg)

            # ---- C/D emission schedule ----
            # wait-hints stop the scheduler from hoisting AR-gated D ops
            # into the middle of C streams (DVE/Pool head-of-line blocking)
            emit_C_ct(0)
            emit_C_ct(1)
            emit_stats2_out(0)
            emit_C_ct(2)
            emit_C_ct(3)
            emit_stats2_out(1)
            with tc.tile_wait_until(10.0):
                emit_D_pair(0)
            with tc.tile_wait_until(10.1):
                emit_D_trans(0)
            with tc.tile_wait_until(10.2):
                emit_D_pair(1)
            with tc.tile_wait_until(10.3):
                emit_D_trans(1)


_NC_CACHE = None
LAST_RES = None


def _get_nc():
    global _NC_CACHE
    if _NC_CACHE is None:
        _NC_CACHE = build_program()
    return _NC_CACHE


def _w2_terms(w2T):
    """Split w2T (H, C) into 4 e4m3 terms; term k stores w2T residual scaled
    by W2S[k]. Returns packed [4*8*128, 2*C] fp8 array for DR lhsT tiles."""
    r = w2T.astype(np.float64)
    terms = []
    for k in range(4):
        t = (r * W2S[k]).astype(ml_dtypes.float8_e4m3)
        terms.append(t)
        r = r - t.astype(np.float64) / W2S[k]
    out = np.empty((4, 8, 128, 2, C), dtype=ml_dtypes.float8_e4m3)
    for k in range(4):
        # [H, C] -> [8 ip, 2 plane, 128 p, C] with i = (2ip+plane)*128+p
        tk = terms[k].reshape(8, 2, 128, C)
        out[k] = tk.transpose(0, 2, 1, 3)
    return out.reshape(4 * 8 * 128, 2 * C)


def kernel(x, w1, g1, b1, w2, g2, b2):
    x = np.asarray(x, dtype=np.float32)
    w1 = np.asarray(w1, dtype=np.float32)
    w2 = np.asarray(w2, dtype=np.float32)
    g1 = np.asarray(g1, dtype=np.float32)
    b1 = np.asarray(b1, dtype=np.float32)
    g2 = np.asarray(g2, dtype=np.float32)
    b2 = np.asarray(b2, dtype=np.float32)

    w1T = np.ascontiguousarray(w1.T)                    # (C, H)
    w1T16 = w1T.astype(np.float16)
    dw1Tb = (w1T - w1T16.astype(np.float32)).astype(ml_dtypes.bfloat16)
    w2p = _w2_terms(np.ascontiguousarray(w2.T))

    xr = x.reshape(T, B_GLOB, NN, C)
    in_maps = []
    for k in range(N_CORES):
        xk = xr[:, k * B_LOC:(k + 1) * B_LOC].reshape(M, C)
        xTk = np.ascontiguousarray(xk.T)                # (C, M)
        xT16 = xTk.astype(np.float16)
        in_maps.append({
            "xT16": xT16,
            "dxTb": (xTk - xT16.astype(np.float32)).astype(ml_dtypes.bfloat16),
            "w1T16": w1T16, "dw1Tb": dw1Tb, "w2p": w2p,
            "g1": g1, "b1": b1, "g2": g2, "b2": b2,
            "chain": np.zeros((1, 128), np.float32),
        })

    nc = _get_nc()
    import os
    trace = bool(int(os.environ.get("KERNEL_TRACE", "0")))
    res = run_bass_kernel_spmd(nc, in_maps, core_ids=list(range(N_CORES)),
                               trace=trace)
    global LAST_RES
    LAST_RES = res

    out = np.empty((T * B_GLOB, NN, C), dtype=np.float32)
    outr = out.reshape(T, B_GLOB, NN, C)
    for k in range(N_CORES):
        qvk = res.results[k]["qv"].astype(np.float32)   # (M, C) in (r, n, c)
        tmp = qvk.reshape(R, C, NN).transpose(0, 2, 1)  # (R, N, C)
        outr[:, k * B_LOC:(k + 1) * B_LOC] = tmp.reshape(T, B_LOC, NN, C)
    return out


# revision 46
# speedup vs baseline: 1.2685x; 1.0048x over previous
"""Trainium2 Bass kernel for nn_Decoder_MLP: Linear->BN->LIF -> Linear->BN->LIF.

Sharding: data-parallel over batch B (TB=T*B=128 rows -> 4 batch items/core,
all T=4 timesteps local). BN batch stats are all-reduced across the 8 cores
(8 group-ARs for layer 1, 2 pair-ARs for layer 2), each hidden under compute.

v2 design (cost-model driven):
- GEMM1 (x@w1T): 3 passes -- fp16(x)@fp16(w1) main + bf16(dx)@fp16(w1) +
  fp16(x)@bf16(dw) crosses. fp16 == RNE-11, so precision ~2^-21. h stays in
  SBUF (no DRAM roundtrip); BN stats all-reduced per 2-ht group so LIF of
  group g runs under the matmuls of group g+1.
- Spikes are exact in fp8e4; stored to DRAM as [m, h] fp8 (the reference's
  "scrambled" reshape is a flat reinterpretation within each r-block, so the
  scrambled read back is an affine AP).
- GEMM2 (s@w2T): w2 split into 4 e4m3 terms (scales 2^5,2^9,2^13,2^17);
  pairs of K-chunks packed into fp8 DoubleRow matmuls (0.5 cyc/row) ->
  same cost as ONE bf16 pass. Terms combine via 2 gpsimd + 1 DVE op per
  tile; the 2^5 overall scale is absorbed exactly by BN (eps scaled 2^10).
"""

import numpy as np
import ml_dtypes

import concourse.bass as bass
import concourse.mybir as mybir
import concourse.tile as tile
from concourse import bacc
from concourse.bass_utils import run_bass_kernel_spmd
from concourse.masks import make_identity

F32 = mybir.dt.float32
F16 = mybir.dt.float16
BF16 = mybir.dt.bfloat16
F8 = mybir.dt.float8e4
ALU = mybir.AluOpType
ACTF = mybir.ActivationFunctionType
DR = mybir.MatmulPerfMode.DoubleRow

N_CORES = 8
T = 4
B_GLOB = 32
B_LOC = B_GLOB // N_CORES          # 4 batch items per core
R = T * B_LOC                      # 16 local (t, b) rows
NN = 196
C = 512
H = 2048
M = R * NN                         # 3136 local rows
M_T = 392                          # m-tile (2 r-rows)
N_MT = M // M_T                    # 8
MB = 112                           # transpose block (3136 = 28*112)
N_MB = M // MB                     # 28
NG = 8                             # ht-groups for layer 1 (2 ht each)
HPG = 2                            # ht per group
EPS = 1e-5
EPS2 = 1e-5 * 1024.0               # layer-2 stats are of o' = 2^5 o
W2S = [2.0 ** 5, 2.0 ** 9, 2.0 ** 13, 2.0 ** 17]


def _emit_lif(nc, lanes, vpool, n_free, n_t=T, n_chunk=1, vp_on_pool=False):
    """BN-apply + LIF, multi-lane and m-chunked so independent dependency
    chains pipeline across engines. lanes: list of (sb_h [128, n_t*n_free],
    sp_out, s_vecs, c_vecs). W_t = W'_{t-1} + 2^t*(scale*h_t + shift);
    spike iff W_t >= 2^{t+1}; W'_t = W_t*(W_t < 2^{t+1})."""
    nf = n_free // n_chunk
    vprev = {}
    for t in range(n_t):
        thr = float(2.0 ** (t + 1))
        for li, (sb_h, sp_out, s_vecs, c_vecs) in enumerate(lanes):
            for ck in range(n_chunk):
                sl = slice(t * n_free + ck * nf, t * n_free + (ck + 1) * nf)
                key = (li, ck)
                if t == 0:
                    v = vpool.tile([128, nf], F32, tag=f"v{li}{ck}")
                    nc.scalar.activation(out=v, in_=sb_h[:, sl],
                                         func=ACTF.Identity,
                                         bias=c_vecs[t], scale=s_vecs[t])
                else:
                    y = vpool.tile([128, nf], F32, tag=f"y{li}{ck}")
                    nc.scalar.activation(out=y, in_=sb_h[:, sl],
                                         func=ACTF.Identity,
                                         bias=c_vecs[t], scale=s_vecs[t])
                    v = vpool.tile([128, nf], F32, tag=f"v{li}{ck}")
                    nc.vector.tensor_tensor(out=v, in0=vprev[key], in1=y,
                                            op=ALU.add)
                nc.gpsimd.tensor_scalar(out=sp_out[:, sl], in0=v,
                                        scalar1=thr, scalar2=None,
                                        op0=ALU.is_ge)
                if t < n_t - 1:
                    vp = vpool.tile([128, nf], F32, tag=f"vp{li}{ck}")
                    nc.vector.scalar_tensor_tensor(out=vp, in0=v, scalar=thr,
                                                   in1=v, op0=ALU.is_lt,
                                                   op1=ALU.mult)
                    vprev[key] = vp


def _emit_stats_to_scales(nc, pool, ar_sb, gamma_sb, beta_sb, w, sfx, eps):
    """ar_sb: (128, 2w) all-reduced [sums-of-means | sums-of-Ex2]. Returns
    per-t (s_vecs, c_vecs) lists of (128, w) tiles: 2^t*scale, 2^t*shift."""
    mean = pool.tile([128, w], F32, tag=f"bnmean{sfx}", name=f"bnmean{sfx}")
    ex2 = pool.tile([128, w], F32, tag=f"bnex2{sfx}", name=f"bnex2{sfx}")
    nc.vector.tensor_scalar(out=mean, in0=ar_sb[:, 0:w], scalar1=1.0 / N_CORES,
                            scalar2=None, op0=ALU.mult)
    nc.vector.tensor_scalar(out=ex2, in0=ar_sb[:, w:2 * w],
                            scalar1=1.0 / N_CORES, scalar2=None, op0=ALU.mult)
    var = pool.tile([128, w], F32, tag=f"bnvar{sfx}", name=f"bnvar{sfx}")
    msq = pool.tile([128, w], F32, tag=f"bnmsq{sfx}", name=f"bnmsq{sfx}")
    nc.vector.tensor_tensor(out=msq, in0=mean, in1=mean, op=ALU.mult)
    nc.vector.tensor_tensor(out=var, in0=ex2, in1=msq, op=ALU.subtract)
    epsb = pool.tile([128, 1], F32, tag=f"bneps{sfx}", name=f"bneps{sfx}")
    nc.vector.memset(epsb, eps)
    std = pool.tile([128, w], F32, tag=f"bnstd{sfx}", name=f"bnstd{sfx}")
    nc.scalar.activation(out=std, in_=var, func=ACTF.Sqrt, bias=epsb, scale=1.0)
    rstd = pool.tile([128, w], F32, tag=f"bnrstd{sfx}", name=f"bnrstd{sfx}")
    nc.vector.reciprocal(out=rstd, in_=std)
    scale = pool.tile([128, w], F32, tag=f"bnscale{sfx}", name=f"bnscale{sfx}")
    nc.vector.tensor_tensor(out=scale, in0=gamma_sb, in1=rstd, op=ALU.mult)
    mscl = pool.tile([128, w], F32, tag=f"bnmscl{sfx}", name=f"bnmscl{sfx}")
    nc.vector.tensor_tensor(out=mscl, in0=mean, in1=scale, op=ALU.mult)
    shift = pool.tile([128, w], F32, tag=f"bnshift{sfx}", name=f"bnshift{sfx}")
    nc.vector.tensor_tensor(out=shift, in0=beta_sb, in1=mscl, op=ALU.subtract)
    s_vecs, c_vecs = [], []
    for t in range(T):
        f = float(2.0 ** t)
        s = pool.tile([128, w], F32, tag=f"bns{t}{sfx}", name=f"bns{t}{sfx}")
        cc = pool.tile([128, w], F32, tag=f"bnc{t}{sfx}", name=f"bnc{t}{sfx}")
        nc.vector.tensor_scalar(out=s, in0=scale, scalar1=f, scalar2=None,
                                op0=ALU.mult)
        nc.vector.tensor_scalar(out=cc, in0=shift, scalar1=f, scalar2=None,
                                op0=ALU.mult)
        s_vecs.append(s)
        c_vecs.append(cc)
    return s_vecs, c_vecs


def build_program():
    nc = bacc.Bacc("TRN2", target_bir_lowering=False, debug=False,
                   num_devices=N_CORES)

    xT16 = nc.dram_tensor("xT16", [C, M], F16, kind="ExternalInput").ap()
    dxTb = nc.dram_tensor("dxTb", [C, M], BF16, kind="ExternalInput").ap()
    w1T16 = nc.dram_tensor("w1T16", [C, H], F16, kind="ExternalInput").ap()
    dw1Tb = nc.dram_tensor("dw1Tb", [C, H], BF16, kind="ExternalInput").ap()
    w2p = nc.dram_tensor("w2p", [4 * 8 * 128, 2 * C], F8,
                         kind="ExternalInput").ap()
    g1 = nc.dram_tensor("g1", [H], F32, kind="ExternalInput").ap()
    b1 = nc.dram_tensor("b1", [H], F32, kind="ExternalInput").ap()
    g2 = nc.dram_tensor("g2", [C], F32, kind="ExternalInput").ap()
    b2 = nc.dram_tensor("b2", [C], F32, kind="ExternalInput").ap()
    qv = nc.dram_tensor("qv", [M, C], BF16, kind="ExternalOutput").ap()
    chain = nc.dram_tensor("chain", [1, 128], F32, kind="ExternalInput").ap()
    chain_o = nc.dram_tensor("chain_o", [1, 128], F32, kind="ExternalOutput").ap()

    with tile.TileContext(nc) as tc:
        _build_body(nc, tc, xT16, dxTb, w1T16, dw1Tb, w2p, g1, b1, g2, b2, qv)
        with tc.tile_pool(name="chainp", bufs=1) as chp:
            cht = chp.tile([1, 128], F32)
            nc.sync.dma_start(out=cht, in_=chain)
            nc.sync.dma_start(out=chain_o, in_=cht)
    nc.compile()
    return nc


def _build_body(nc, tc, xT16, dxTb, w1T16, dw1Tb, w2p, g1, b1, g2, b2, qv):
    from contextlib import ExitStack

    with ExitStack() as octx:
        persist = octx.enter_context(tc.tile_pool(name="persist", bufs=1))
        dram = octx.enter_context(tc.tile_pool(name="dram", bufs=1, space="DRAM"))

        id_bf = persist.tile([128, 128], BF16)
        g1_sb = persist.tile([128, 16], F32)
        b1_sb = persist.tile([128, 16], F32)
        g2_sb = persist.tile([128, 4], F32)
        b2_sb = persist.tile([128, 4], F32)

        def emit_globals():
            # deferred: not needed until B(0)/trans; keeps the SP queue
            # clear for the first w1/x loads
            make_identity(nc, id_bf)
            nc.sync.dma_start(out=g1_sb, in_=g1.rearrange("(a b) -> b a", b=128))
            nc.sync.dma_start(out=b1_sb, in_=b1.rearrange("(a b) -> b a", b=128))
            nc.sync.dma_start(out=g2_sb, in_=g2.rearrange("(a b) -> b a", b=128))
            nc.sync.dma_start(out=b2_sb, in_=b2.rearrange("(a b) -> b a", b=128))

        spk = dram.tile([M, H], F8)
        ar1_in = dram.tile([NG, 128, 4], F32)
        ar1_outs = [dram.tile([128, 4], F32, addr_space="Shared",
                              tag=f"ar1o{g}", name=f"ar1o{g}") for g in range(NG)]
        ar2_in = dram.tile([2, 128, 4], F32)
        ar2_outs = [dram.tile([128, 4], F32, addr_space="Shared",
                              tag=f"ar2o{i}", name=f"ar2o{i}") for i in range(2)]

        scales1 = {}
        scales2 = {}
        sp_tiles = {}
        st_tiles = {}
        w1sbs = {}
        h_tiles = {}

        # ============== A+B superphase: 8 ht-groups pipelined ==============
        with tc.tile_pool(name="px", bufs=1) as px, \
             tc.tile_pool(name="pw1", bufs=2) as pw1, \
             tc.tile_pool(name="ph", bufs=2) as ph, \
             tc.tile_pool(name="pA_st", bufs=1) as past, \
             tc.tile_pool(name="pA_ps", bufs=6, space="PSUM") as paps, \
             tc.tile_pool(name="pB_v", bufs=3) as pbv, \
             tc.tile_pool(name="pB_sp", bufs=2) as pbsp, \
             tc.tile_pool(name="pB_stg", bufs=2) as pbstg, \
             tc.tile_pool(name="pB_ps", bufs=2, space="PSUM") as pbps:

            x16_lo = px.tile([128, 2, M], F16)
            x16_hi = px.tile([128, 2, M], F16)
            dxb_sb = px.tile([128, 4, M], BF16)

            def x16_sl(c, msl):
                return (x16_lo if c < 2 else x16_hi)[:, c % 2, msl]

            def emit_x_load(part):           # 4 parts of 2 mt each
                msl = slice(part * 784, (part + 1) * 784)
                for half, xt in ((0, x16_lo), (1, x16_hi)):
                    nc.sync.dma_start(
                        out=xt[:, :, msl],
                        in_=bass.AP(tensor=xT16.tensor,
                                    offset=xT16.offset + part * 784
                                    + half * 2 * 128 * M,
                                    ap=[[M, 128], [128 * M, 2], [1, 784]]))
                nc.sync.dma_start(
                    out=dxb_sb[:, :, msl],
                    in_=bass.AP(tensor=dxTb.tensor,
                                offset=dxTb.offset + part * 784,
                                ap=[[M, 128], [128 * M, 4], [1, 784]]))

            def emit_w1_load(g):
                csl = slice(g * HPG * 128, (g + 1) * HPG * 128)
                w16 = pw1.tile([128, 4, HPG * 128], F16, tag="w16",
                               name=f"w16_{g}")
                wbf = pw1.tile([128, 4, HPG * 128], BF16, tag="wbf",
                               name=f"wbf_{g}")
                nc.sync.dma_start(
                    out=w16,
                    in_=bass.AP(tensor=w1T16.tensor,
                                offset=w1T16.offset + g * HPG * 128,
                                ap=[[H, 128], [128 * H, 4], [1, HPG * 128]]))
                nc.sync.dma_start(
                    out=wbf,
                    in_=bass.AP(tensor=dw1Tb.tensor,
                                offset=dw1Tb.offset + g * HPG * 128,
                                ap=[[H, 128], [128 * H, 4], [1, HPG * 128]]))
                w1sbs[g] = (w16, wbf)

            def emit_A_group(g):
                w16, wbf = w1sbs[g]
                hts = [g * HPG + i for i in range(HPG)]
                for ht in hts:
                    st_tiles[ht] = past.tile([128, N_MT, 6], F32,
                                             tag=f"st{ht}", name=f"st{ht}")
                    h_tiles[ht] = ph.tile([128, M], F32, tag=f"h{ht % HPG}",
                                          name=f"h_{ht}")
                for mt in range(N_MT):
                    msl = slice(mt * M_T, (mt + 1) * M_T)
                    for hl, ht in enumerate(hts):
                        hsl = slice(hl * 128, (hl + 1) * 128)
                        ps = paps.tile([128, M_T], F32, tag="ps")
                        for c in range(4):
                            nc.tensor.matmul(ps, w16[:, c, hsl],
                                             x16_sl(c, msl),
                                             start=(c == 0), stop=False)
                        for c in range(4):
                            nc.tensor.matmul(ps, w16[:, c, hsl],
                                             dxb_sb[:, c, msl],
                                             start=False, stop=False)
                        for c in range(4):
                            nc.tensor.matmul(ps, wbf[:, c, hsl],
                                             x16_sl(c, msl),
                                             start=False, stop=(c == 3))
                        # alternate drain engine so one engine's LIF burst
                        # never stalls PSUM recycling
                        if mt % 2 == 0:
                            nc.scalar.activation(out=h_tiles[ht][:, msl],
                                                 in_=ps, func=ACTF.Copy)
                        else:
                            nc.vector.tensor_copy(h_tiles[ht][:, msl], ps)
                        nc.vector.bn_stats(out=st_tiles[ht][:, mt, :],
                                           in_=h_tiles[ht][:, msl])

            def emit_stats_out(g):
                hts = [g * HPG + i for i in range(HPG)]
                stats = persist.tile([128, 4], F32, tag=f"stats1_{g}",
                                     name=f"stats1_{g}")
                for hl, ht in enumerate(hts):
                    mv = past.tile([128, 2], F32, tag="mv", name=f"mv{ht}")
                    nc.vector.bn_aggr(out=mv, in_=st_tiles[ht])
                    nc.vector.tensor_copy(stats[:, hl:hl + 1], mv[:, 0:1])
                    nc.vector.scalar_tensor_tensor(
                        out=stats[:, HPG + hl:HPG + hl + 1],
                        in0=mv[:, 0:1], scalar=mv[:, 0:1], in1=mv[:, 1:2],
                        op0=ALU.mult, op1=ALU.add)
                nc.sync.dma_start(out=ar1_in[g], in_=stats)
                nc.gpsimd.collective_compute(
                    "AllReduce", ALU.add, replica_groups=[list(range(N_CORES))],
                    ins=[ar1_in[g].opt()], outs=[ar1_outs[g].opt()])

            def emit_scales1(g):
                ar = persist.tile([128, 4], F32, tag=f"ar1_{g}",
                                  name=f"ar1_{g}")
                nc.sync.dma_start(out=ar, in_=ar1_outs[g])
                scales1[g] = _emit_stats_to_scales(
                    nc, persist, ar,
                    g1_sb[:, g * HPG:(g + 1) * HPG],
                    b1_sb[:, g * HPG:(g + 1) * HPG], HPG, f"L1g{g}", EPS)

            def emit_B_group(g):
                emit_scales1(g)
                s1v, c1v = scales1[g]
                for hl in range(HPG):
                    ht = g * HPG + hl
                    sp = pbsp.tile([128, M], BF16, tag=f"s{ht % 4}",
                                   name=f"sp_{ht}")
                    _emit_lif(nc, [(h_tiles[ht], sp,
                                    [s[:, hl:hl + 1] for s in s1v],
                                    [cv[:, hl:hl + 1] for cv in c1v])],
                              pbv, NN * B_LOC)
                    sp_tiles[ht] = sp

            def emit_B_trans(hts, hcol0):
                """Transpose spikes of `hts` (columns hcol0..+128*len) into
                spk DRAM; len(hts) in {2, 4}."""
                nh = len(hts)
                w = nh * 128
                sps = [sp_tiles[ht] for ht in hts]
                stg = None
                for mb in range(N_MB):
                    pstf = pbps.tile([MB, 512], BF16, tag="pst")
                    pst = pstf[:, 0:w] if w < 512 else pstf
                    for hh in range(nh):
                        nc.tensor.matmul(
                            pst[:, hh * 128:(hh + 1) * 128],
                            sps[hh][:, mb * MB:(mb + 1) * MB],
                            id_bf, is_transpose=True,
                            start=(hh == 0), stop=(hh == nh - 1))
                    if mb % 7 == 0:
                        stg = pbstg.tile([MB, 7, w], F8, tag=f"stg{w}")
                    if mb % 2 == 0:
                        nc.scalar.activation(out=stg[:, mb % 7, :], in_=pst,
                                             func=ACTF.Copy)
                    else:
                        nc.vector.tensor_copy(stg[:, mb % 7, :], pst)
                    if mb % 7 == 6:
                        m0 = (mb - 6) * MB
                        nc.sync.dma_start(
                            out=bass.AP(tensor=spk.tensor,
                                        offset=spk.offset + m0 * H + hcol0,
                                        ap=[[H, MB], [MB * H, 7], [1, w]]),
                            in_=stg)

            # ---- A/B emission schedule ----
            emit_w1_load(0)
            emit_x_load(0)
            emit_w1_load(1)
            emit_globals()
            for part in range(1, 4):
                emit_x_load(part)
            for g in range(NG):
                emit_A_group(g)
                emit_stats_out(g)
                if g + 1 < NG:
                    emit_w1_load(g + 1)
                if g >= 1:
                    emit_B_group(g - 1)       # LIF of g-1 under A(g+1) emission
                if g == 3:
                    emit_B_trans([0, 1, 2, 3], 0)
                if g == 5:
                    emit_B_trans([4, 5, 6, 7], 512)
            emit_B_group(NG - 1)
            emit_B_trans([8, 9, 10, 11], 1024)      # AR(7)-window fill
            emit_B_trans([12, 13], 1536)
            emit_B_trans([14, 15], 1792)            # gated on B(7) LIF

        # ============== C+D superphase ==============
        with tc.tile_pool(name="pw2", bufs=1) as pw2, \
             tc.tile_pool(name="prhs", bufs=1) as prhs, \
             tc.tile_pool(name="po", bufs=2) as po, \
             tc.tile_pool(name="pC_st", bufs=1) as pcst, \
             tc.tile_pool(name="pC_tmp", bufs=2) as ptmp, \
             tc.tile_pool(name="pC_ps23", bufs=1, space="PSUM") as pp23, \
             tc.tile_pool(name="pC_ps01", bufs=2, space="PSUM") as pp01, \
             tc.tile_pool(name="pD_v", bufs=1) as pdv, \
             tc.tile_pool(name="pD_sp", bufs=2) as pdsp, \
             tc.tile_pool(name="pD_stg", bufs=2) as pdstg, \
             tc.tile_pool(name="pD_ps", bufs=2, space="PSUM") as pdps:

            w2sb = []
            for k in range(4):
                wt = pw2.tile([128, 8, 2, C], F8, tag=f"w2_{k}",
                              name=f"w2_{k}")
                nc.sync.dma_start(
                    out=wt,
                    in_=bass.AP(tensor=w2p.tensor,
                                offset=w2p.offset + k * 8 * 128 * 2 * C,
                                ap=[[2 * C, 128], [128 * 2 * C, 8], [1, 2 * C]]))
                w2sb.append(wt)

            rhs = [[None, None] for _ in range(8)]   # rhs[ip][r-half]
            for hf in range(2):
                for ip in range(8):
                    rt = prhs.tile([128, 2, 8, NN], F8, tag=f"rhs{ip}_{hf}",
                                   name=f"rhs{ip}_{hf}")
                    for pl in range(2):
                        nc.sync.dma_start(
                            out=rt[:, pl, :, :],
                            in_=bass.AP(tensor=spk.tensor,
                                        offset=spk.offset
                                        + (ip * 2 + pl) * 128 * NN
                                        + hf * 8 * NN * H,
                                        ap=[[NN, 128], [NN * H, 8], [1, NN]]))
                    rhs[ip][hf] = rt

            o_tiles = {}
            st2_tiles = {}
            sp2 = {}

            def emit_C_ct(ct):
                o_tiles[ct] = po.tile([128, M], F32, tag=f"o{ct % 2}",
                                      name=f"o_{ct}")
                st2_tiles[ct] = pcst.tile([128, N_MT, 6], F32,
                                          tag=f"st2_{ct}", name=f"st2_{ct}")
                csl = slice(ct * 128, (ct + 1) * 128)
                for mt in range(N_MT):
                    msl = slice(mt * M_T, (mt + 1) * M_T)
                    pss = {}
                    for k in (3, 2, 1, 0):
                        pool = pp23 if k >= 2 else pp01
                        ps = pool.tile([128, M_T], F32, tag=f"p{k}")
                        for ip in range(8):
                            rr = rhs[ip][mt // 4][:, :, 2 * (mt % 4):
                                                  2 * (mt % 4) + 2, :] \
                                .rearrange("p a b c -> p a (b c)")
                            nc.tensor.matmul(ps, w2sb[k][:, ip, :, csl], rr,
                                             start=(ip == 0), stop=(ip == 7),
                                             perf_mode=DR)
                        pss[k] = ps
                        if k == 3:
                            # ACT prescale: only one PSUM input allowed per op
                            c3 = ptmp.tile([128, M_T], F32, tag="c3")
                            nc.scalar.activation(out=c3, in_=pss[3],
                                                 func=ACTF.Copy,
                                                 scale=2.0 ** -4)
                        if k == 2:
                            t1 = ptmp.tile([128, M_T], F32, tag="t1")
                            nc.vector.tensor_tensor(out=t1, in0=c3,
                                                    in1=pss[2], op=ALU.add)
                    t2 = ptmp.tile([128, M_T], F32, tag="t2")
                    nc.vector.scalar_tensor_tensor(
                        out=t2, in0=t1, scalar=2.0 ** -4, in1=pss[1],
                        op0=ALU.mult, op1=ALU.add)
                    osl = o_tiles[ct][:, msl]
                    nc.vector.scalar_tensor_tensor(
                        out=osl, in0=t2, scalar=2.0 ** -4, in1=pss[0],
                        op0=ALU.mult, op1=ALU.add)
                    nc.vector.bn_stats(out=st2_tiles[ct][:, mt, :], in_=osl)

            def emit_stats2_out(pair):
                cts = [2 * pair, 2 * pair + 1]
                stats = persist.tile([128, 4], F32, tag=f"stats2_{pair}",
                                     name=f"stats2_{pair}")
                for i, ct in enumerate(cts):
                    mv2 = pcst.tile([128, 2], F32, tag="mv2", name=f"mv2{ct}")
                    nc.vector.bn_aggr(out=mv2, in_=st2_tiles[ct])
                    nc.vector.tensor_copy(stats[:, i:i + 1], mv2[:, 0:1])
                    nc.vector.scalar_tensor_tensor(
                        out=stats[:, 2 + i:2 + i + 1],
                        in0=mv2[:, 0:1], scalar=mv2[:, 0:1], in1=mv2[:, 1:2],
                        op0=ALU.mult, op1=ALU.add)
                nc.sync.dma_start(out=ar2_in[pair], in_=stats)
                nc.gpsimd.collective_compute(
                    "AllReduce", ALU.add, replica_groups=[list(range(N_CORES))],
                    ins=[ar2_in[pair].opt()], outs=[ar2_outs[pair].opt()])

            def emit_D_pair(pair):
                # allocate from the C-combine pool: the slot-reuse WAR edge
                # stops the scheduler hoisting this AR-gated chain into the
                # middle of C (DVE/Pool head-of-line blocking)
                ar = ptmp.tile([128, 4], F32, tag="t2", name=f"ar2_{pair}")
                nc.sync.dma_start(out=ar, in_=ar2_outs[pair])
                scales2[pair] = _emit_stats_to_scales(
                    nc, persist, ar, g2_sb[:, 2 * pair:2 * pair + 2],
                    b2_sb[:, 2 * pair:2 * pair + 2], 2, f"L2p{pair}", EPS2)
                s2v, c2v = scales2[pair]
                lanes = []
                for i in range(2):
                    ct = 2 * pair + i
                    sp = pdsp.tile([128, M], BF16, tag=f"sp2_{i}",
                                   name=f"sp2_{ct}")
                    lanes.append((o_tiles[ct], sp,
                                  [s[:, i:i + 1] for s in s2v],
                                  [cv[:, i:i + 1] for cv in c2v]))
                    sp2[ct] = sp
                _emit_lif(nc, lanes, pdv, NN * B_LOC, n_chunk=2)

            def emit_D_trans(pair):
                cts = [2 * pair, 2 * pair + 1]
                stg = None
                for mb in range(N_MB):
                    pst = pdps.tile([MB, 256], BF16, tag="pst2")
                    for i, ct in enumerate(cts):
                        nc.tensor.matmul(
                            pst[:, i * 128:(i + 1) * 128],
                            sp2[ct][:, mb * MB:(mb + 1) * MB],
                            id_bf, is_transpose=True,
                            start=(i == 0), stop=(i == 1))
                    if mb % 7 == 0:
                        stg = pdstg.tile([MB, 7, 256], BF16, tag="stg2")
                    if mb % 2 == 0:
                        nc.scalar.activation(out=stg[:, mb % 7, :], in_=pst,
                                             func=ACTF.Copy)
                    else:
                        nc.vector.tensor_copy(stg[:, mb % 7, :], pst)
                    if mb % 7 == 6:
                        m0 = (mb - 6) * MB
                        nc.sync.dma_start(
                            out=bass.AP(tensor=qv.tensor,
                                        offset=qv.offset + m0 * C + pair * 256,
                                        ap=[[C, MB], [MB * C, 7], [1, 256]]),
                            in_=stg)

            # ---- C/D emission schedule ----
            # wait-hints stop the scheduler from hoisting AR-gated D ops
            # into the middle of C streams (DVE/Pool head-of-line blocking)
            emit_C_ct(0)
            emit_C_ct(1)
            emit_stats2_out(0)
            emit_C_ct(2)
            emit_C_ct(3)
            emit_stats2_out(1)
            with tc.tile_wait_until(10.0):
                emit_D_pair(0)
            with tc.tile_wait_until(10.1):
                emit_D_trans(0)
            with tc.tile_wait_until(10.2):
                emit_D_pair(1)
            with tc.tile_wait_until(10.3):
                emit_D_trans(1)


_NC_CACHE = None
LAST_RES = None


def _get_nc():
    global _NC_CACHE
    if _NC_CACHE is None:
        _NC_CACHE = build_program()
    return _NC_CACHE


def _w2_terms(w2T):
    """Split w2T (H, C) into 4 e4m3 terms; term k stores w2T residual scaled
    by W2S[k]. Returns packed [4*8*128, 2*C] fp8 array for DR lhsT tiles."""
    r = w2T.astype(np.float64)
    terms = []
    for k in range(4):
        t = (r * W2S[k]).astype(ml_dtypes.float8_e4m3)
        terms.append(t)
        r = r - t.astype(np.float64) / W2S[k]
    out = np.empty((4, 8, 128, 2, C), dtype=ml_dtypes.float8_e4m3)
    for k in range(4):
        # [H, C] -> [8 ip, 2 plane, 128 p, C] with i = (2ip+plane)*128+p
        tk = terms[k].reshape(8, 2, 128, C)
        out[k] = tk.transpose(0, 2, 1, 3)
    return out.reshape(4 * 8 * 128, 2 * C)


def kernel(x, w1, g1, b1, w2, g2, b2):
    x = np.asarray(x, dtype=np.float32)
    w1 = np.asarray(w1, dtype=np.float32)
    w2 = np.asarray(w2, dtype=np.float32)
    g1 = np.asarray(g1, dtype=np.float32)
    b1 = np.asarray(b1, dtype=np.float32)
    g2 = np.asarray(g2, dtype=np.float32)
    b2 = np.asarray(b2, dtype=np.float32)

    w1T = np.ascontiguousarray(w1.T)                    # (C, H)
    w1T16 = w1T.astype(np.float16)
    dw1Tb = (w1T - w1T16.astype(np.float32)).astype(ml_dtypes.bfloat16)
    w2p = _w2_terms(np.ascontiguousarray(w2.T))

    xr = x.reshape(T, B_GLOB, NN, C)
    in_maps = []
    for k in range(N_CORES):
        xk = xr[:, k * B_LOC:(k + 1) * B_LOC].reshape(M, C)
        xTk = np.ascontiguousarray(xk.T)                # (C, M)
        xT16 = xTk.astype(np.float16)
        in_maps.append({
            "xT16": xT16,
            "dxTb": (xTk - xT16.astype(np.float32)).astype(ml_dtypes.bfloat16),
            "w1T16": w1T16, "dw1Tb": dw1Tb, "w2p": w2p,
            "g1": g1, "b1": b1, "g2": g2, "b2": b2,
            "chain": np.zeros((1, 128), np.float32),
        })

    nc = _get_nc()
    import os
    trace = bool(int(os.environ.get("KERNEL_TRACE", "0")))
    res = run_bass_kernel_spmd(nc, in_maps, core_ids=list(range(N_CORES)),
                               trace=trace)
    global LAST_RES
    LAST_RES = res

    out = np.empty((T * B_GLOB, NN, C), dtype=np.float32)
    outr = out.reshape(T, B_GLOB, NN, C)
    for k in range(N_CORES):
        qvk = res.results[k]["qv"].astype(np.float32)   # (M, C) in (r, n, c)
        tmp = qvk.reshape(R, C, NN).transpose(0, 2, 1)  # (R, N, C)
        outr[:, k * B_LOC:(k + 1) * B_LOC] = tmp.reshape(T, B_LOC, NN, C)
    return out


# revision 48
# speedup vs baseline: 1.2708x; 1.0018x over previous
"""Trainium2 Bass kernel for nn_Decoder_MLP: Linear->BN->LIF -> Linear->BN->LIF.

Sharding: data-parallel over batch B (TB=T*B=128 rows -> 4 batch items/core,
all T=4 timesteps local). BN batch stats are all-reduced across the 8 cores
(8 group-ARs for layer 1, 2 pair-ARs for layer 2), each hidden under compute.

v2 design (cost-model driven):
- GEMM1 (x@w1T): 3 passes -- fp16(x)@fp16(w1) main + bf16(dx)@fp16(w1) +
  fp16(x)@bf16(dw) crosses. fp16 == RNE-11, so precision ~2^-21. h stays in
  SBUF (no DRAM roundtrip); BN stats all-reduced per 2-ht group so LIF of
  group g runs under the matmuls of group g+1.
- Spikes are exact in fp8e4; stored to DRAM as [m, h] fp8 (the reference's
  "scrambled" reshape is a flat reinterpretation within each r-block, so the
  scrambled read back is an affine AP).
- GEMM2 (s@w2T): w2 split into 4 e4m3 terms (scales 2^5,2^9,2^13,2^17);
  pairs of K-chunks packed into fp8 DoubleRow matmuls (0.5 cyc/row) ->
  same cost as ONE bf16 pass. Terms combine via 2 gpsimd + 1 DVE op per
  tile; the 2^5 overall scale is absorbed exactly by BN (eps scaled 2^10).
"""

import numpy as np
import ml_dtypes

import concourse.bass as bass
import concourse.mybir as mybir
import concourse.tile as tile
from concourse import bacc
from concourse.bass_utils import run_bass_kernel_spmd
from concourse.masks import make_identity

F32 = mybir.dt.float32
F16 = mybir.dt.float16
BF16 = mybir.dt.bfloat16
F8 = mybir.dt.float8e4
ALU = mybir.AluOpType
ACTF = mybir.ActivationFunctionType
DR = mybir.MatmulPerfMode.DoubleRow

N_CORES = 8
T = 4
B_GLOB = 32
B_LOC = B_GLOB // N_CORES          # 4 batch items per core
R = T * B_LOC                      # 16 local (t, b) rows
NN = 196
C = 512
H = 2048
M = R * NN                         # 3136 local rows
M_T = 392                          # m-tile (2 r-rows)
N_MT = M // M_T                    # 8
MB = 112                           # transpose block (3136 = 28*112)
N_MB = M // MB                     # 28
NG = 8                             # ht-groups for layer 1 (2 ht each)
HPG = 2                            # ht per group
EPS = 1e-5
EPS2 = 1e-5 * 1024.0               # layer-2 stats are of o' = 2^5 o
W2S = [2.0 ** 5, 2.0 ** 9, 2.0 ** 13, 2.0 ** 17]


def _emit_lif(nc, lanes, vpool, n_free, n_t=T, n_chunk=1, vp_on_pool=False):
    """BN-apply + LIF, multi-lane and m-chunked so independent dependency
    chains pipeline across engines. lanes: list of (sb_h [128, n_t*n_free],
    sp_out, s_vecs, c_vecs). W_t = W'_{t-1} + 2^t*(scale*h_t + shift);
    spike iff W_t >= 2^{t+1}; W'_t = W_t*(W_t < 2^{t+1})."""
    nf = n_free // n_chunk
    vprev = {}
    for t in range(n_t):
        thr = float(2.0 ** (t + 1))
        for li, (sb_h, sp_out, s_vecs, c_vecs) in enumerate(lanes):
            for ck in range(n_chunk):
                sl = slice(t * n_free + ck * nf, t * n_free + (ck + 1) * nf)
                key = (li, ck)
                if t == 0:
                    v = vpool.tile([128, nf], F32, tag=f"v{li}{ck}")
                    nc.scalar.activation(out=v, in_=sb_h[:, sl],
                                         func=ACTF.Identity,
                                         bias=c_vecs[t], scale=s_vecs[t])
                else:
                    y = vpool.tile([128, nf], F32, tag=f"y{li}{ck}")
                    nc.scalar.activation(out=y, in_=sb_h[:, sl],
                                         func=ACTF.Identity,
                                         bias=c_vecs[t], scale=s_vecs[t])
                    v = vpool.tile([128, nf], F32, tag=f"v{li}{ck}")
                    nc.vector.tensor_tensor(out=v, in0=vprev[key], in1=y,
                                            op=ALU.add)
                nc.gpsimd.tensor_scalar(out=sp_out[:, sl], in0=v,
                                        scalar1=thr, scalar2=None,
                                        op0=ALU.is_ge)
                if t < n_t - 1:
                    vp = vpool.tile([128, nf], F32, tag=f"vp{li}{ck}")
                    nc.vector.scalar_tensor_tensor(out=vp, in0=v, scalar=thr,
                                                   in1=v, op0=ALU.is_lt,
                                                   op1=ALU.mult)
                    vprev[key] = vp


def _emit_stats_to_scales(nc, pool, ar_sb, gamma_sb, beta_sb, w, sfx, eps):
    """ar_sb: (128, 2w) all-reduced [sums-of-means | sums-of-Ex2]. Returns
    per-t (s_vecs, c_vecs) lists of (128, w) tiles: 2^t*scale, 2^t*shift."""
    mean = pool.tile([128, w], F32, tag=f"bnmean{sfx}", name=f"bnmean{sfx}")
    ex2 = pool.tile([128, w], F32, tag=f"bnex2{sfx}", name=f"bnex2{sfx}")
    nc.vector.tensor_scalar(out=mean, in0=ar_sb[:, 0:w], scalar1=1.0 / N_CORES,
                            scalar2=None, op0=ALU.mult)
    nc.vector.tensor_scalar(out=ex2, in0=ar_sb[:, w:2 * w],
                            scalar1=1.0 / N_CORES, scalar2=None, op0=ALU.mult)
    var = pool.tile([128, w], F32, tag=f"bnvar{sfx}", name=f"bnvar{sfx}")
    msq = pool.tile([128, w], F32, tag=f"bnmsq{sfx}", name=f"bnmsq{sfx}")
    nc.vector.tensor_tensor(out=msq, in0=mean, in1=mean, op=ALU.mult)
    nc.vector.tensor_tensor(out=var, in0=ex2, in1=msq, op=ALU.subtract)
    epsb = pool.tile([128, 1], F32, tag=f"bneps{sfx}", name=f"bneps{sfx}")
    nc.vector.memset(epsb, eps)
    std = pool.tile([128, w], F32, tag=f"bnstd{sfx}", name=f"bnstd{sfx}")
    nc.scalar.activation(out=std, in_=var, func=ACTF.Sqrt, bias=epsb, scale=1.0)
    rstd = pool.tile([128, w], F32, tag=f"bnrstd{sfx}", name=f"bnrstd{sfx}")
    nc.vector.reciprocal(out=rstd, in_=std)
    scale = pool.tile([128, w], F32, tag=f"bnscale{sfx}", name=f"bnscale{sfx}")
    nc.vector.tensor_tensor(out=scale, in0=gamma_sb, in1=rstd, op=ALU.mult)
    mscl = pool.tile([128, w], F32, tag=f"bnmscl{sfx}", name=f"bnmscl{sfx}")
    nc.vector.tensor_tensor(out=mscl, in0=mean, in1=scale, op=ALU.mult)
    shift = pool.tile([128, w], F32, tag=f"bnshift{sfx}", name=f"bnshift{sfx}")
    nc.vector.tensor_tensor(out=shift, in0=beta_sb, in1=mscl, op=ALU.subtract)
    s_vecs, c_vecs = [], []
    for t in range(T):
        f = float(2.0 ** t)
        s = pool.tile([128, w], F32, tag=f"bns{t}{sfx}", name=f"bns{t}{sfx}")
        cc = pool.tile([128, w], F32, tag=f"bnc{t}{sfx}", name=f"bnc{t}{sfx}")
        nc.vector.tensor_scalar(out=s, in0=scale, scalar1=f, scalar2=None,
                                op0=ALU.mult)
        nc.vector.tensor_scalar(out=cc, in0=shift, scalar1=f, scalar2=None,
                                op0=ALU.mult)
        s_vecs.append(s)
        c_vecs.append(cc)
    return s_vecs, c_vecs


def build_program():
    nc = bacc.Bacc("TRN2", target_bir_lowering=False, debug=False,
                   num_devices=N_CORES)

    xT16 = nc.dram_tensor("xT16", [C, M], F16, kind="ExternalInput").ap()
    dxTb = nc.dram_tensor("dxTb", [C, M], BF16, kind="ExternalInput").ap()
    w1T16 = nc.dram_tensor("w1T16", [C, H], F16, kind="ExternalInput").ap()
    dw1Tb = nc.dram_tensor("dw1Tb", [C, H], BF16, kind="ExternalInput").ap()
    w2p = nc.dram_tensor("w2p", [4 * 8 * 128, 2 * C], F8,
                         kind="ExternalInput").ap()
    g1 = nc.dram_tensor("g1", [H], F32, kind="ExternalInput").ap()
    b1 = nc.dram_tensor("b1", [H], F32, kind="ExternalInput").ap()
    g2 = nc.dram_tensor("g2", [C], F32, kind="ExternalInput").ap()
    b2 = nc.dram_tensor("b2", [C], F32, kind="ExternalInput").ap()
    qv = nc.dram_tensor("qv", [M, C], BF16, kind="ExternalOutput").ap()
    chain = nc.dram_tensor("chain", [1, 128], F32, kind="ExternalInput").ap()
    chain_o = nc.dram_tensor("chain_o", [1, 128], F32, kind="ExternalOutput").ap()

    with tile.TileContext(nc) as tc:
        _build_body(nc, tc, xT16, dxTb, w1T16, dw1Tb, w2p, g1, b1, g2, b2, qv)
        with tc.tile_pool(name="chainp", bufs=1) as chp:
            cht = chp.tile([1, 128], F32)
            nc.sync.dma_start(out=cht, in_=chain)
            nc.sync.dma_start(out=chain_o, in_=cht)
    nc.compile()
    return nc


def _build_body(nc, tc, xT16, dxTb, w1T16, dw1Tb, w2p, g1, b1, g2, b2, qv):
    from contextlib import ExitStack

    with ExitStack() as octx:
        persist = octx.enter_context(tc.tile_pool(name="persist", bufs=1))
        dram = octx.enter_context(tc.tile_pool(name="dram", bufs=1, space="DRAM"))

        id_bf = persist.tile([128, 128], BF16)
        g1_sb = persist.tile([128, 16], F32)
        b1_sb = persist.tile([128, 16], F32)
        g2_sb = persist.tile([128, 4], F32)
        b2_sb = persist.tile([128, 4], F32)

        def emit_globals():
            # deferred: not needed until B(0)/trans; keeps the SP queue
            # clear for the first w1/x loads
            make_identity(nc, id_bf)
            nc.sync.dma_start(out=g1_sb, in_=g1.rearrange("(a b) -> b a", b=128))
            nc.sync.dma_start(out=b1_sb, in_=b1.rearrange("(a b) -> b a", b=128))
            nc.sync.dma_start(out=g2_sb, in_=g2.rearrange("(a b) -> b a", b=128))
            nc.sync.dma_start(out=b2_sb, in_=b2.rearrange("(a b) -> b a", b=128))

        spk = dram.tile([M, H], F8)
        ar1_in = dram.tile([NG, 128, 4], F32)
        ar1_outs = [dram.tile([128, 4], F32, addr_space="Shared",
                              tag=f"ar1o{g}", name=f"ar1o{g}") for g in range(NG)]
        ar2_in = dram.tile([2, 128, 4], F32)
        ar2_outs = [dram.tile([128, 4], F32, addr_space="Shared",
                              tag=f"ar2o{i}", name=f"ar2o{i}") for i in range(2)]

        scales1 = {}
        scales2 = {}
        sp_tiles = {}
        st_tiles = {}
        w1sbs = {}
        h_tiles = {}

        # ============== A+B superphase: 8 ht-groups pipelined ==============
        with tc.tile_pool(name="px", bufs=1) as px, \
             tc.tile_pool(name="pw1", bufs=2) as pw1, \
             tc.tile_pool(name="ph", bufs=2) as ph, \
             tc.tile_pool(name="pA_st", bufs=1) as past, \
             tc.tile_pool(name="pA_ps", bufs=6, space="PSUM") as paps, \
             tc.tile_pool(name="pB_v", bufs=3) as pbv, \
             tc.tile_pool(name="pB_sp", bufs=2) as pbsp, \
             tc.tile_pool(name="pB_stg", bufs=2) as pbstg, \
             tc.tile_pool(name="pB_ps", bufs=2, space="PSUM") as pbps:

            x16_lo = px.tile([128, 2, M], F16)
            x16_hi = px.tile([128, 2, M], F16)
            dxb_sb = px.tile([128, 4, M], BF16)

            def x16_sl(c, msl):
                return (x16_lo if c < 2 else x16_hi)[:, c % 2, msl]

            def emit_x_load(part, w=784):    # parts of w columns
                msl = slice(part * w, part * w + w)
                for half, xt in ((0, x16_lo), (1, x16_hi)):
                    nc.sync.dma_start(
                        out=xt[:, :, msl],
                        in_=bass.AP(tensor=xT16.tensor,
                                    offset=xT16.offset + part * w
                                    + half * 2 * 128 * M,
                                    ap=[[M, 128], [128 * M, 2], [1, w]]))
                nc.sync.dma_start(
                    out=dxb_sb[:, :, msl],
                    in_=bass.AP(tensor=dxTb.tensor,
                                offset=dxTb.offset + part * w,
                                ap=[[M, 128], [128 * M, 4], [1, w]]))

            def emit_w1_load(g):
                csl = slice(g * HPG * 128, (g + 1) * HPG * 128)
                w16 = pw1.tile([128, 4, HPG * 128], F16, tag="w16",
                               name=f"w16_{g}")
                wbf = pw1.tile([128, 4, HPG * 128], BF16, tag="wbf",
                               name=f"wbf_{g}")
                nc.sync.dma_start(
                    out=w16,
                    in_=bass.AP(tensor=w1T16.tensor,
                                offset=w1T16.offset + g * HPG * 128,
                                ap=[[H, 128], [128 * H, 4], [1, HPG * 128]]))
                nc.sync.dma_start(
                    out=wbf,
                    in_=bass.AP(tensor=dw1Tb.tensor,
                                offset=dw1Tb.offset + g * HPG * 128,
                                ap=[[H, 128], [128 * H, 4], [1, HPG * 128]]))
                w1sbs[g] = (w16, wbf)

            def emit_A_group(g):
                w16, wbf = w1sbs[g]
                hts = [g * HPG + i for i in range(HPG)]
                for ht in hts:
                    st_tiles[ht] = past.tile([128, N_MT, 6], F32,
                                             tag=f"st{ht}", name=f"st{ht}")
                    h_tiles[ht] = ph.tile([128, M], F32, tag=f"h{ht % HPG}",
                                          name=f"h_{ht}")
                for mt in range(N_MT):
                    msl = slice(mt * M_T, (mt + 1) * M_T)
                    for hl, ht in enumerate(hts):
                        hsl = slice(hl * 128, (hl + 1) * 128)
                        ps = paps.tile([128, M_T], F32, tag="ps")
                        for c in range(4):
                            nc.tensor.matmul(ps, w16[:, c, hsl],
                                             x16_sl(c, msl),
                                             start=(c == 0), stop=False)
                        for c in range(4):
                            nc.tensor.matmul(ps, w16[:, c, hsl],
                                             dxb_sb[:, c, msl],
                                             start=False, stop=False)
                        for c in range(4):
                            nc.tensor.matmul(ps, wbf[:, c, hsl],
                                             x16_sl(c, msl),
                                             start=False, stop=(c == 3))
                        # alternate drain engine so one engine's LIF burst
                        # never stalls PSUM recycling
                        if mt % 2 == 0:
                            nc.scalar.activation(out=h_tiles[ht][:, msl],
                                                 in_=ps, func=ACTF.Copy)
                        else:
                            nc.vector.tensor_copy(h_tiles[ht][:, msl], ps)
                        nc.vector.bn_stats(out=st_tiles[ht][:, mt, :],
                                           in_=h_tiles[ht][:, msl])

            def emit_stats_out(g):
                hts = [g * HPG + i for i in range(HPG)]
                stats = persist.tile([128, 4], F32, tag=f"stats1_{g}",
                                     name=f"stats1_{g}")
                for hl, ht in enumerate(hts):
                    mv = past.tile([128, 2], F32, tag="mv", name=f"mv{ht}")
                    nc.vector.bn_aggr(out=mv, in_=st_tiles[ht])
                    nc.vector.tensor_copy(stats[:, hl:hl + 1], mv[:, 0:1])
                    nc.vector.scalar_tensor_tensor(
                        out=stats[:, HPG + hl:HPG + hl + 1],
                        in0=mv[:, 0:1], scalar=mv[:, 0:1], in1=mv[:, 1:2],
                        op0=ALU.mult, op1=ALU.add)
                nc.sync.dma_start(out=ar1_in[g], in_=stats)
                nc.gpsimd.collective_compute(
                    "AllReduce", ALU.add, replica_groups=[list(range(N_CORES))],
                    ins=[ar1_in[g].opt()], outs=[ar1_outs[g].opt()])

            def emit_scales1(g):
                ar = persist.tile([128, 4], F32, tag=f"ar1_{g}",
                                  name=f"ar1_{g}")
                nc.sync.dma_start(out=ar, in_=ar1_outs[g])
                scales1[g] = _emit_stats_to_scales(
                    nc, persist, ar,
                    g1_sb[:, g * HPG:(g + 1) * HPG],
                    b1_sb[:, g * HPG:(g + 1) * HPG], HPG, f"L1g{g}", EPS)

            def emit_B_group(g):
                emit_scales1(g)
                s1v, c1v = scales1[g]
                for hl in range(HPG):
                    ht = g * HPG + hl
                    sp = pbsp.tile([128, M], BF16, tag=f"s{ht % 4}",
                                   name=f"sp_{ht}")
                    _emit_lif(nc, [(h_tiles[ht], sp,
                                    [s[:, hl:hl + 1] for s in s1v],
                                    [cv[:, hl:hl + 1] for cv in c1v])],
                              pbv, NN * B_LOC)
                    sp_tiles[ht] = sp

            def emit_B_trans(hts, hcol0):
                """Transpose spikes of `hts` (columns hcol0..+128*len) into
                spk DRAM; len(hts) in {2, 4}."""
                nh = len(hts)
                w = nh * 128
                sps = [sp_tiles[ht] for ht in hts]
                stg = None
                for mb in range(N_MB):
                    pstf = pbps.tile([MB, 512], BF16, tag="pst")
                    pst = pstf[:, 0:w] if w < 512 else pstf
                    for hh in range(nh):
                        nc.tensor.matmul(
                            pst[:, hh * 128:(hh + 1) * 128],
                            sps[hh][:, mb * MB:(mb + 1) * MB],
                            id_bf, is_transpose=True,
                            start=(hh == 0), stop=(hh == nh - 1))
                    if mb % 7 == 0:
                        stg = pbstg.tile([MB, 7, w], F8, tag=f"stg{w}")
                    if mb % 2 == 0:
                        nc.scalar.activation(out=stg[:, mb % 7, :], in_=pst,
                                             func=ACTF.Copy)
                    else:
                        nc.vector.tensor_copy(stg[:, mb % 7, :], pst)
                    if mb % 7 == 6:
                        m0 = (mb - 6) * MB
                        nc.sync.dma_start(
                            out=bass.AP(tensor=spk.tensor,
                                        offset=spk.offset + m0 * H + hcol0,
                                        ap=[[H, MB], [MB * H, 7], [1, w]]),
                            in_=stg)

            # ---- A/B emission schedule ----
            emit_w1_load(0)
            emit_x_load(0, 392)
            emit_x_load(1, 392)
            emit_w1_load(1)
            emit_globals()
            for part in range(1, 4):
                emit_x_load(part)
            for g in range(NG):
                emit_A_group(g)
                emit_stats_out(g)
                if g + 1 < NG:
                    emit_w1_load(g + 1)
                if g >= 1:
                    emit_B_group(g - 1)       # LIF of g-1 under A(g+1) emission
                if g == 3:
                    emit_B_trans([0, 1, 2, 3], 0)
                if g == 5:
                    emit_B_trans([4, 5, 6, 7], 512)
            emit_B_group(NG - 1)
            emit_B_trans([8, 9, 10, 11], 1024)      # AR(7)-window fill
            emit_B_trans([12, 13], 1536)
            emit_B_trans([14, 15], 1792)            # gated on B(7) LIF

        # ============== C+D superphase ==============
        with tc.tile_pool(name="pw2", bufs=1) as pw2, \
             tc.tile_pool(name="prhs", bufs=1) as prhs, \
             tc.tile_pool(name="po", bufs=2) as po, \
             tc.tile_pool(name="pC_st", bufs=1) as pcst, \
             tc.tile_pool(name="pC_tmp", bufs=2) as ptmp, \
             tc.tile_pool(name="pC_ps23", bufs=1, space="PSUM") as pp23, \
             tc.tile_pool(name="pC_ps01", bufs=2, space="PSUM") as pp01, \
             tc.tile_pool(name="pD_v", bufs=1) as pdv, \
             tc.tile_pool(name="pD_sp", bufs=2) as pdsp, \
             tc.tile_pool(name="pD_stg", bufs=2) as pdstg, \
             tc.tile_pool(name="pD_ps", bufs=2, space="PSUM") as pdps:

            w2sb = []
            for k in range(4):
                wt = pw2.tile([128, 8, 2, C], F8, tag=f"w2_{k}",
                              name=f"w2_{k}")
                nc.sync.dma_start(
                    out=wt,
                    in_=bass.AP(tensor=w2p.tensor,
                                offset=w2p.offset + k * 8 * 128 * 2 * C,
                                ap=[[2 * C, 128], [128 * 2 * C, 8], [1, 2 * C]]))
                w2sb.append(wt)

            rhs = [[None, None] for _ in range(8)]   # rhs[ip][r-half]
            for hf in range(2):
                for ip in range(8):
                    rt = prhs.tile([128, 2, 8, NN], F8, tag=f"rhs{ip}_{hf}",
                                   name=f"rhs{ip}_{hf}")
                    for pl in range(2):
                        nc.sync.dma_start(
                            out=rt[:, pl, :, :],
                            in_=bass.AP(tensor=spk.tensor,
                                        offset=spk.offset
                                        + (ip * 2 + pl) * 128 * NN
                                        + hf * 8 * NN * H,
                                        ap=[[NN, 128], [NN * H, 8], [1, NN]]))
                    rhs[ip][hf] = rt

            o_tiles = {}
            st2_tiles = {}
            sp2 = {}

            def emit_C_ct(ct):
                o_tiles[ct] = po.tile([128, M], F32, tag=f"o{ct % 2}",
                                      name=f"o_{ct}")
                st2_tiles[ct] = pcst.tile([128, N_MT, 6], F32,
                                          tag=f"st2_{ct}", name=f"st2_{ct}")
                csl = slice(ct * 128, (ct + 1) * 128)
                for mt in range(N_MT):
                    msl = slice(mt * M_T, (mt + 1) * M_T)
                    pss = {}
                    for k in (3, 2, 1, 0):
                        pool = pp23 if k >= 2 else pp01
                        ps = pool.tile([128, M_T], F32, tag=f"p{k}")
                        for ip in range(8):
                            rr = rhs[ip][mt // 4][:, :, 2 * (mt % 4):
                                                  2 * (mt % 4) + 2, :] \
                                .rearrange("p a b c -> p a (b c)")
                            nc.tensor.matmul(ps, w2sb[k][:, ip, :, csl], rr,
                                             start=(ip == 0), stop=(ip == 7),
                                             perf_mode=DR)
                        pss[k] = ps
                        if k == 3:
                            # ACT prescale: only one PSUM input allowed per op
                            c3 = ptmp.tile([128, M_T], F32, tag="c3")
                            nc.scalar.activation(out=c3, in_=pss[3],
                                                 func=ACTF.Copy,
                                                 scale=2.0 ** -4)
                        if k == 2:
                            t1 = ptmp.tile([128, M_T], F32, tag="t1")
                            nc.vector.tensor_tensor(out=t1, in0=c3,
                                                    in1=pss[2], op=ALU.add)
                    t2 = ptmp.tile([128, M_T], F32, tag="t2")
                    nc.vector.scalar_tensor_tensor(
                        out=t2, in0=t1, scalar=2.0 ** -4, in1=pss[1],
                        op0=ALU.mult, op1=ALU.add)
                    osl = o_tiles[ct][:, msl]
                    nc.vector.scalar_tensor_tensor(
                        out=osl, in0=t2, scalar=2.0 ** -4, in1=pss[0],
                        op0=ALU.mult, op1=ALU.add)
                    nc.vector.bn_stats(out=st2_tiles[ct][:, mt, :], in_=osl)

            def emit_stats2_out(pair):
                cts = [2 * pair, 2 * pair + 1]
                stats = persist.tile([128, 4], F32, tag=f"stats2_{pair}",
                                     name=f"stats2_{pair}")
                for i, ct in enumerate(cts):
                    mv2 = pcst.tile([128, 2], F32, tag="mv2", name=f"mv2{ct}")
                    nc.vector.bn_aggr(out=mv2, in_=st2_tiles[ct])
                    nc.vector.tensor_copy(stats[:, i:i + 1], mv2[:, 0:1])
                    nc.vector.scalar_tensor_tensor(
                        out=stats[:, 2 + i:2 + i + 1],
                        in0=mv2[:, 0:1], scalar=mv2[:, 0:1], in1=mv2[:, 1:2],
                        op0=ALU.mult, op1=ALU.add)
                nc.sync.dma_start(out=ar2_in[pair], in_=stats)
                nc.gpsimd.collective_compute(
                    "AllReduce", ALU.add, replica_groups=[list(range(N_CORES))],
                    ins=[ar2_in[pair].opt()], outs=[ar2_outs[pair].opt()])

            def emit_D_pair(pair):
                # allocate from the C-combine pool: the slot-reuse WAR edge
                # stops the scheduler hoisting this AR-gated chain into the
                # middle of C (DVE/Pool head-of-line blocking)
                ar = ptmp.tile([128, 4], F32, tag="t2", name=f"ar2_{pair}")
                nc.sync.dma_start(out=ar, in_=ar2_outs[pair])
                scales2[pair] = _emit_stats_to_scales(
                    nc, persist, ar, g2_sb[:, 2 * pair:2 * pair + 2],
                    b2_sb[:, 2 * pair:2 * pair + 2], 2, f"L2p{pair}", EPS2)
                s2v, c2v = scales2[pair]
                lanes = []
                for i in range(2):
                    ct = 2 * pair + i
                    sp = pdsp.tile([128, M], BF16, tag=f"sp2_{i}",
                                   name=f"sp2_{ct}")
                    lanes.append((o_tiles[ct], sp,
                                  [s[:, i:i + 1] for s in s2v],
                                  [cv[:, i:i + 1] for cv in c2v]))
                    sp2[ct] = sp
                _emit_lif(nc, lanes, pdv, NN * B_LOC, n_chunk=2)

            def emit_D_trans(pair):
                cts = [2 * pair, 2 * pair + 1]
                stg = None
                for mb in range(N_MB):
                    pst = pdps.tile([MB, 256], BF16, tag="pst2")
                    for i, ct in enumerate(cts):
                        nc.tensor.matmul(
                            pst[:, i * 128:(i + 1) * 128],
                            sp2[ct][:, mb * MB:(mb + 1) * MB],
                            id_bf, is_transpose=True,
                            start=(i == 0), stop=(i == 1))
                    if mb % 7 == 0:
                        stg = pdstg.tile([MB, 7, 256], BF16, tag="stg2")
                    if mb % 2 == 0:
                        nc.scalar.activation(out=stg[:, mb % 7, :], in_=pst,
                                             func=ACTF.Copy)
                    else:
                        nc.vector.tensor_copy(stg[:, mb % 7, :], pst)
                    if mb % 7 == 6:
                        m0 = (mb - 6) * MB
                        nc.sync.dma_start(
                            out=bass.AP(tensor=qv.tensor,
                                        offset=qv.offset + m0 * C + pair * 256,
                                        ap=[[C, MB], [MB * C, 7], [1, 256]]),
                            in_=stg)

            # ---- C/D emission schedule ----
            # wait-hints stop the scheduler from hoisting AR-gated D ops
            # into the middle of C streams (DVE/Pool head-of-line blocking)
            emit_C_ct(0)
            emit_C_ct(1)
            emit_stats2_out(0)
            emit_C_ct(2)
            emit_C_ct(3)
            emit_stats2_out(1)
            with tc.tile_wait_until(10.0):
                emit_D_pair(0)
            with tc.tile_wait_until(10.1):
                emit_D_trans(0)
            with tc.tile_wait_until(10.2):
                emit_D_pair(1)
            with tc.tile_wait_until(10.3):
                emit_D_trans(1)


_NC_CACHE = None
LAST_RES = None


def _get_nc():
    global _NC_CACHE
    if _NC_CACHE is None:
        _NC_CACHE = build_program()
    return _NC_CACHE


def _w2_terms(w2T):
    """Split w2T (H, C) into 4 e4m3 terms; term k stores w2T residual scaled
    by W2S[k]. Returns packed [4*8*128, 2*C] fp8 array for DR lhsT tiles."""
    r = w2T.astype(np.float64)
    terms = []
    for k in range(4):
        t = (r * W2S[k]).astype(ml_dtypes.float8_e4m3)
        terms.append(t)
        r = r - t.astype(np.float64) / W2S[k]
    out = np.empty((4, 8, 128, 2, C), dtype=ml_dtypes.float8_e4m3)
    for k in range(4):
        # [H, C] -> [8 ip, 2 plane, 128 p, C] with i = (2ip+plane)*128+p
        tk = terms[k].reshape(8, 2, 128, C)
        out[k] = tk.transpose(0, 2, 1, 3)
    return out.reshape(4 * 8 * 128, 2 * C)


def kernel(x, w1, g1, b1, w2, g2, b2):
    x = np.asarray(x, dtype=np.float32)
    w1 = np.asarray(w1, dtype=np.float32)
    w2 = np.asarray(w2, dtype=np.float32)
    g1 = np.asarray(g1, dtype=np.float32)
    b1 = np.asarray(b1, dtype=np.float32)
    g2 = np.asarray(g2, dtype=np.float32)
    b2 = np.asarray(b2, dtype=np.float32)

    w1T = np.ascontiguousarray(w1.T)                    # (C, H)
    w1T16 = w1T.astype(np.float16)
    dw1Tb = (w1T - w1T16.astype(np.float32)).astype(ml_dtypes.bfloat16)
    w2p = _w2_terms(np.ascontiguousarray(w2.T))

    xr = x.reshape(T, B_GLOB, NN, C)
    in_maps = []
    for k in range(N_CORES):
        xk = xr[:, k * B_LOC:(k + 1) * B_LOC].reshape(M, C)
        xTk = np.ascontiguousarray(xk.T)                # (C, M)
        xT16 = xTk.astype(np.float16)
        in_maps.append({
            "xT16": xT16,
            "dxTb": (xTk - xT16.astype(np.float32)).astype(ml_dtypes.bfloat16),
            "w1T16": w1T16, "dw1Tb": dw1Tb, "w2p": w2p,
            "g1": g1, "b1": b1, "g2": g2, "b2": b2,
            "chain": np.zeros((1, 128), np.float32),
        })

    nc = _get_nc()
    import os
    trace = bool(int(os.environ.get("KERNEL_TRACE", "0")))
    res = run_bass_kernel_spmd(nc, in_maps, core_ids=list(range(N_CORES)),
                               trace=trace)
    global LAST_RES
    LAST_RES = res

    out = np.empty((T * B_GLOB, NN, C), dtype=np.float32)
    outr = out.reshape(T, B_GLOB, NN, C)
    for k in range(N_CORES):
        qvk = res.results[k]["qv"].astype(np.float32)   # (M, C) in (r, n, c)
        tmp = qvk.reshape(R, C, NN).transpose(0, 2, 1)  # (R, N, C)
        outr[:, k * B_LOC:(k + 1) * B_LOC] = tmp.reshape(T, B_LOC, NN, C)
    return out
